# revision 42
# baseline (speedup 1.0000x reference)
"""Trainium2 Bass kernel for nn_DSCBR (gnn_message_passing).

Strategy (8 NeuronCores, SPMD, dest-sharded):
- Host prunes both propagation graphs by backward slicing from the loss batch
  (only rows that feed the final losses are computed), then compacts each
  layer's destination space; compact rows are round-robin sharded.
- Layer-1 SpMM sources come from the INPUT feature tables, so the host
  pre-gathers them into per-core edge-ordered payload streams (pure indexing;
  all FP math stays on device).  The device streams payloads contiguously
  (HWDGE), multiplies by edge values, and segment-sums via selection-matrix
  matmuls accumulated in PSUM bank tiles.
- Layer-2/agg SpMM sources are runtime tables; gathered per edge with
  dma_gather spread over 4 SWDGE queues.
- f1 tables are all-gathered in per-block sub-collectives so layer-2 can
  start on block 0 while later blocks are still in flight.
- Losses (BPR + two contrastive views) computed batch-sharded + AllReduce.
"""
import os
import sys
import types

sys.path.insert(0, "/opt/trn_rl_repo")

import numpy as np

import concourse.bass as bass
import concourse.bacc as bacc
import concourse.mybir as mybir
import concourse.tile as tile
from concourse.bass_utils import run_bass_kernel_spmd
from concourse.masks import make_identity

P = 128
NCORES = 8
SRC_WIN = 32768
BLK_SLOTS = SRC_WIN // NCORES   # 4096 per-core slots per AG block
GI_MAX = 2048
D = 64
NU, NI, NB = 100000, 50000, 20000
BATCH = 2048
F32 = mybir.dt.float32
I32 = mybir.dt.int32
I16 = mybir.dt.int16
BF = mybir.dt.bfloat16
AF = mybir.ActivationFunctionType
ALU = mybir.AluOpType


# ---------------------------------------------------------------- host prep

def _pad_ids(real, n_space, mult):
    """real: sorted unique ids. Append complement ids to a multiple of mult."""
    need = (-len(real)) % mult
    if need == 0:
        return np.asarray(real, np.int64)
    m = np.ones(n_space, bool)
    m[real] = False
    pad = np.flatnonzero(m)[:need]
    assert len(pad) == need, "no room to pad id set"
    return np.concatenate([np.asarray(real, np.int64), pad])


def _pad_concat(base, extra, n_space, mult):
    arr = np.concatenate([np.asarray(base, np.int64), np.asarray(extra, np.int64)])
    need = (-len(arr)) % mult
    if need == 0:
        return arr
    m = np.ones(n_space, bool)
    m[arr] = False
    pad = np.flatnonzero(m)[:need]
    assert len(pad) == need
    return np.concatenate([arr, pad])


def _posmap(ids, n_space):
    g = np.full(n_space, -1, np.int64)
    g[ids] = np.arange(len(ids))
    return g


def _blk_sizes(R):
    """Per-core block slot counts (multiples of 128), blocks of <=BLK_SLOTS."""
    out = []
    left = R
    while left > 0:
        out.append(min(BLK_SLOTS, left))
        left -= out[-1]
    return out


def _runs(mask):
    """Maximal [a,b) runs of True in a 1-d bool array."""
    out = []
    a = None
    for i, v in enumerate(mask):
        if v and a is None:
            a = i
        elif not v and a is not None:
            out.append((a, i))
            a = None
    if a is not None:
        out.append((a, len(mask)))
    return out


def build_stream(rows_pos, swin, srcv, vals, R, nsrc, ncores=NCORES):
    """Build per-core dest-sharded edge streams with (swin, win-pair, lrow)
    grouping, 128-edge chunks padded to the max count over cores.

    Returns a dict with per-core streams and the shared chunk program."""
    nwp = R // (2 * P)
    rows_pos = np.asarray(rows_pos, np.int64)
    swin = np.asarray(swin, np.int64)
    core = rows_pos % ncores
    slot = rows_pos // ncores
    wp = slot // (2 * P)
    lrow = slot % (2 * P)
    order = np.lexsort((lrow, wp, swin, core))
    c_s, s_s, w_s, l_s = core[order], swin[order], wp[order], lrow[order]
    sv_s, v_s = np.asarray(srcv)[order], np.asarray(vals)[order]

    counts = np.zeros((ncores, nsrc, nwp), np.int64)
    np.add.at(counts, (c_s, s_s, w_s), 1)
    countsA = np.zeros((ncores, nsrc, nwp), np.int64)
    mA = l_s < P
    np.add.at(countsA, (c_s[mA], s_s[mA], w_s[mA]), 1)
    maxc = counts.max(axis=0)
    nchunks = (maxc + P - 1) // P                     # [nsrc, nwp]
    tch = int(nchunks.sum())
    grp_choff = np.concatenate([[0], np.cumsum(nchunks.ravel())])[:-1].reshape(nsrc, nwp)

    key = (c_s * nsrc + s_s) * nwp + w_s
    starts = np.searchsorted(key, np.arange(ncores * nsrc * nwp))
    ends = np.searchsorted(key, np.arange(ncores * nsrc * nwp) + 1)

    srcv_st = np.zeros((ncores, tch * P), np.int64)
    lrA = np.full((ncores, tch * P), 300.0, np.float32)
    lrB = np.full((ncores, tch * P), 300.0, np.float32)
    val_st = np.zeros((ncores, tch * P), np.float32)
    ch_s = np.zeros(tch, np.int64)
    ch_w = np.zeros(tch, np.int64)
    hasA = np.zeros(tch, bool)
    hasB = np.zeros(tch, bool)
    for s in range(nsrc):
        for w in range(nwp):
            ncw = int(nchunks[s, w])
            if ncw == 0:
                continue
            off = int(grp_choff[s, w])
            ch_s[off:off + ncw] = s
            ch_w[off:off + ncw] = w
            for c in range(ncores):
                k = (c * nsrc + s) * nwp + w
                a, b = int(starts[k]), int(ends[k])
                n = b - a
                if n == 0:
                    continue
                pos = off * P
                srcv_st[c, pos:pos + n] = sv_s[a:b]
                lr = l_s[a:b]
                lrA[c, pos:pos + n] = np.where(lr < P, lr, 300.0)
                lrB[c, pos:pos + n] = np.where(lr >= P, lr - P, 300.0)
                val_st[c, pos:pos + n] = v_s[a:b]
                nAc = int(countsA[c, s, w])
                kA = (nAc + P - 1) // P
                hasA[off:off + kA] = True
                if n > nAc:
                    hasB[off + nAc // P: off + (n + P - 1) // P] = True

    nwin = R // P
    firstq = np.full(2 * nwp, -1, np.int64)
    lastq = np.full(2 * nwp, -1, np.int64)
    for q in range(tch):
        for half, has in ((0, hasA[q]), (1, hasB[q])):
            if has:
                wf = 2 * ch_w[q] + half
                if firstq[wf] < 0:
                    firstq[wf] = q
                lastq[wf] = q
    memset_wf = [wf for wf in range(nwin) if firstq[wf] < 0]

    # per-(segment, wp, half) start/stop flags and drain ops.
    # PSUM accumulation groups must not interleave within a bank, so each
    # (s, w, half) is its own group in its own tile; drains copy on the
    # half's first touched segment and add on later ones.
    stA = np.zeros(tch, bool)
    spA = np.zeros(tch, bool)
    stB = np.zeros(tch, bool)
    spB = np.zeros(tch, bool)
    addA = np.zeros(tch, bool)   # at spA chunk: accumulate into raw, not copy
    addB = np.zeros(tch, bool)
    seen_wf = set()
    for s in range(nsrc):
        for w in range(nwp):
            ncw = int(nchunks[s, w])
            if ncw == 0:
                continue
            off = int(grp_choff[s, w])
            qs = np.arange(off, off + ncw)
            for half, hm, stX, spX, adX in ((0, hasA, stA, spA, addA),
                                            (1, hasB, stB, spB, addB)):
                qa = qs[hm[qs]]
                if len(qa) == 0:
                    continue
                stX[qa[0]] = True
                spX[qa[-1]] = True
                wf = 2 * w + half
                if wf in seen_wf:
                    adX[qa[-1]] = True
                seen_wf.add(wf)

    batches = []
    for s in range(nsrc):
        lo = int(grp_choff[s, 0])
        hi = int(grp_choff[s, nwp - 1] + nchunks[s, nwp - 1]) if nwp else lo
        q0 = lo
        while q0 < hi:
            nch = min(GI_MAX // P, hi - q0)
            batches.append((s, q0, nch))
            q0 += nch

    return dict(srcv=srcv_st, lrA=lrA, lrB=lrB, val=val_st,
                ch_s=ch_s, ch_w=ch_w, hasA=hasA, hasB=hasB,
                firstq=firstq, lastq=lastq, memset_wf=memset_wf,
                stA=stA, spA=spA, stB=stB, spB=spB, addA=addA, addB=addB,
                batches=batches, tch=tch, nwp=nwp, R=R, nsrc=nsrc)


def wrap_idx16(flat):
    # index i -> partition i%16, col i//16; replicated x8 down partitions
    return np.ascontiguousarray(np.tile(flat.reshape(-1, 16).T.astype(np.int16), (8, 1)))


def idx_cols_i32(flat):
    # [n] -> [128, n/128] int32; col k = rows [128k, 128k+128)
    n = flat.shape[0]
    assert n % P == 0
    return np.ascontiguousarray(flat.reshape(-1, P).T.astype(np.int32))


def _table_pos(g, R):
    """compact position -> row in the core-slab-major all-gathered table."""
    return (g % NCORES) * R + g // NCORES


def _blk_pos(g, sizes):
    """compact position -> (block, idx within block) for block-major tables."""
    c, s = g % NCORES, g // NCORES
    k = s // BLK_SLOTS
    sz = np.asarray(sizes, np.int64)[k]
    return k, c * sz + (s - k * BLK_SLOTS)


def preprocess(inputs, ncores=NCORES):
    u = np.asarray(inputs["users_feature"], np.float32)
    it = np.asarray(inputs["items_feature"], np.float32)
    b = np.asarray(inputs["bundles_feature"], np.float32)
    f0_il = np.concatenate([u, it], 0)
    f0_bl = np.concatenate([u, b], 0)
    N1, N2 = NU + NI, NU + NB

    il_row = np.asarray(inputs["il_row"], np.int64)
    il_col = np.asarray(inputs["il_col"], np.int64)
    il_val = np.asarray(inputs["il_val"], np.float32)
    bl_row = np.asarray(inputs["bl_row"], np.int64)
    bl_col = np.asarray(inputs["bl_col"], np.int64)
    bl_val = np.asarray(inputs["bl_val"], np.float32)
    agg_row = np.asarray(inputs["agg_row"], np.int64)
    agg_col = np.asarray(inputs["agg_col"], np.int64)
    agg_val = np.asarray(inputs["agg_val"], np.float32)
    users = np.asarray(inputs["users"], np.int64)
    bundles = np.asarray(inputs["bundles"], np.int64)

    # ---- active sets (backward slice from the loss batch)
    mB = np.zeros(NB, bool)
    mB[bundles.ravel()] = True
    BstarP = _pad_ids(np.flatnonzero(mB), NB, 2048)
    gB = _posmap(BstarP, NB)
    keep_ag = mB[agg_row]
    items = np.unique(agg_col[keep_ag])
    uuniq = np.unique(users)

    # il graph
    S_acc_il = np.union1d(uuniq, NU + items)
    S_acc_ilP = _pad_ids(S_acc_il, N1, 2048)
    g_acc_il = _posmap(S_acc_ilP, N1)
    acc_mask_il = np.zeros(N1, bool)
    acc_mask_il[S_acc_il] = True
    keep2_il = acc_mask_il[il_row]
    S2_il = np.unique(il_col[keep2_il])
    inP = np.zeros(N1, bool)
    inP[S_acc_ilP] = True
    F1_ilP = _pad_concat(S_acc_ilP, S2_il[~inP[S2_il]], N1, 2048)
    g_f1_il = _posmap(F1_ilP, N1)
    f1_mask_il = acc_mask_il.copy()
    f1_mask_il[S2_il] = True
    keep1_il = f1_mask_il[il_row]

    # bl graph
    S_acc_bl = np.union1d(uuniq, NU + BstarP)
    S_acc_blP = _pad_ids(S_acc_bl, N2, 2048)
    g_acc_bl = _posmap(S_acc_blP, N2)
    acc_mask_bl = np.zeros(N2, bool)
    acc_mask_bl[S_acc_bl] = True
    keep2_bl = acc_mask_bl[bl_row]
    S2_bl = np.unique(bl_col[keep2_bl])
    inP2 = np.zeros(N2, bool)
    inP2[S_acc_blP] = True
    F1_blP = _pad_concat(S_acc_blP, S2_bl[~inP2[S2_bl]], N2, 2048)
    g_f1_bl = _posmap(F1_blP, N2)
    f1_mask_bl = acc_mask_bl.copy()
    f1_mask_bl[S2_bl] = True
    keep1_bl = f1_mask_bl[bl_row]

    R_acc1 = len(S_acc_ilP) // ncores
    R_f11 = len(F1_ilP) // ncores
    R_acc2 = len(S_acc_blP) // ncores
    R_f12 = len(F1_blP) // ncores
    RB = len(BstarP) // ncores
    blks_f11 = _blk_sizes(R_f11)
    blks_f12 = _blk_sizes(R_f12)

    # ---- edge streams
    il1 = build_stream(g_f1_il[il_row[keep1_il]], np.zeros(int(keep1_il.sum()), np.int64),
                       il_col[keep1_il], il_val[keep1_il], R_f11, 1)
    bl1 = build_stream(g_f1_bl[bl_row[keep1_bl]], np.zeros(int(keep1_bl.sum()), np.int64),
                       bl_col[keep1_bl], bl_val[keep1_bl], R_f12, 1)

    p2 = g_f1_il[il_col[keep2_il]]
    k2, i2 = _blk_pos(p2, blks_f11)
    il2 = build_stream(g_acc_il[il_row[keep2_il]], k2, i2,
                       il_val[keep2_il], R_acc1, len(blks_f11))
    p2b = g_f1_bl[bl_col[keep2_bl]]
    k2b, i2b = _blk_pos(p2b, blks_f12)
    bl2 = build_stream(g_acc_bl[bl_row[keep2_bl]], k2b, i2b,
                       bl_val[keep2_bl], R_acc2, len(blks_f12))

    pag = _table_pos(g_acc_il[NU + agg_col[keep_ag]], R_acc1)
    nsrc_ag = (ncores * R_acc1 + SRC_WIN - 1) // SRC_WIN
    ag = build_stream(gB[agg_row[keep_ag]], pag // SRC_WIN, pag % SRC_WIN,
                      agg_val[keep_ag], RB, nsrc_ag)

    # ---- L1 payloads (host pre-gather of INPUT features; edge-ordered)
    import ml_dtypes
    def payload(st, f0):
        out = []
        for c in range(ncores):
            arr = f0[st["srcv"][c]]                     # [tch*128, D]
            arr = arr.reshape(st["tch"], P, D).transpose(1, 0, 2).reshape(P, st["tch"] * D)
            out.append(np.ascontiguousarray(arr.astype(ml_dtypes.bfloat16)))
        return out
    il1_pay = payload(il1, f0_il)
    bl1_pay = payload(bl1, f0_bl)

    # ---- L1 selection matrices (host-built one-hots, packed per half)
    def sel_streams(st):
        tch = st["tch"]
        packs = {}
        for half, key in ((0, "lrA"), (1, "lrB")):
            hm = st["hasA"] if half == 0 else st["hasB"]
            qs = np.flatnonzero(hm)
            pos = np.full(tch, -1, np.int64)
            pos[qs] = np.arange(len(qs))
            mats = []
            for c in range(ncores):
                lr = st[key][c].reshape(tch, P)[qs]          # [nq, 128]
                m = np.zeros((len(qs), P, P), np.float32)
                ii, pp_ = np.nonzero(lr < 299.0)
                m[ii, pp_, lr[ii, pp_].astype(np.int64)] = 1.0
                mats.append(np.ascontiguousarray(
                    m.transpose(1, 0, 2).reshape(P, len(qs) * P).astype(ml_dtypes.bfloat16)))
            packs[half] = (mats, pos, len(qs))
        st["selpack"] = packs
    sel_streams(il1)
    sel_streams(bl1)

    # ---- initial acc rows (f0 at compact acc rows, per core)
    f0_acc_il = [np.ascontiguousarray(f0_il[S_acc_ilP[c::ncores]]) for c in range(ncores)]
    f0_acc_bl = [np.ascontiguousarray(f0_bl[S_acc_blP[c::ncores]]) for c in range(ncores)]

    # ---- loss indices
    loss = {}
    bsh = BATCH // ncores
    for c in range(ncores):
        sl = slice(c * bsh, (c + 1) * bsh)
        loss[c] = dict(
            u_il=idx_cols_i32(_table_pos(g_acc_il[users[sl]], R_acc1)),
            u_bl=idx_cols_i32(_table_pos(g_acc_bl[users[sl]], R_acc2)),
            b_il0=idx_cols_i32(_table_pos(gB[bundles[sl, 0]], RB)),
            b_il1=idx_cols_i32(_table_pos(gB[bundles[sl, 1]], RB)),
            b_bl0=idx_cols_i32(_table_pos(g_acc_bl[bundles[sl, 0] + NU], R_acc2)),
            b_bl1=idx_cols_i32(_table_pos(g_acc_bl[bundles[sl, 1] + NU], R_acc2)),
        )
    aug_u_bl = idx_cols_i32(_table_pos(g_acc_bl[users], R_acc2))
    aug_b0_bl = idx_cols_i32(_table_pos(g_acc_bl[bundles[:, 0] + NU], R_acc2))
    aug_b0_il = idx_cols_i32(_table_pos(gB[bundles[:, 0]], RB))

    return dict(streams=dict(il1=il1, bl1=bl1, il2=il2, bl2=bl2, ag=ag),
                pay=dict(il1=il1_pay, bl1=bl1_pay),
                f0_acc=dict(il=f0_acc_il, bl=f0_acc_bl),
                loss=loss, aug_u_bl=aug_u_bl, aug_b0_bl=aug_b0_bl,
                aug_b0_il=aug_b0_il,
                dims=dict(R_acc1=R_acc1, R_f11=R_f11, R_acc2=R_acc2,
                          R_f12=R_f12, RB=RB, blks_f11=blks_f11,
                          blks_f12=blks_f12, nsrc_ag=nsrc_ag))


# ---------------------------------------------------------------- bass build

class Ctx:
    pass


NPB = 5  # psum bank tiles (8 windows each)


def emit_graph(cx, name, st, src_tables, pay_dram, idx_dram, raw_put, ag_after=None,
               sel_dram=None, sel_pos=None):
    """Emit one SpMM layer.

    src_tables: list of table APs per source window (gather mode), or None.
    pay_dram: payload dram tensor (stream mode), or None.
    raw_put(wf) -> (tile, col_slice) drain destination.
    ag_after: optional dict {batch_index: callable} to emit sub-AG right
    after that batch (pipelined collectives).
    """
    nc = cx.nc
    lrA_sb, lrB_sb, val_sb = cx.meta[name]
    for wf in st["memset_wf"]:
        t, sl = raw_put(wf)
        nc.vector.memset(t[:, sl], 0.0)

    hasA, hasB = st["hasA"], st["hasB"]
    ch_w = st["ch_w"]
    stA, spA, stB, spB = st["stA"], st["spA"], st["stB"], st["spB"]
    addA, addB = st["addA"], st["addB"]
    open_ps = {}

    for bi, (s, q0, nch) in enumerate(st["batches"]):
        gi = nch * P
        if pay_dram is not None:
            g = cx.gsp.tile([P, (GI_MAX // P) * D], BF, tag="gs", name="gs")
            nc.sync.dma_start(out=g[:, :nch * D], in_=pay_dram[:, q0 * D:(q0 + nch) * D])
        else:
            idx_t = cx.idxp.tile([128, GI_MAX // 16], I16, tag="gidx", name="gidx")
            nc.sync.dma_start(out=idx_t[:, :gi // 16],
                              in_=idx_dram[:, q0 * 8:(q0 + nch) * 8])
            g = cx.gp.tile([P, (GI_MAX // P) * D], F32, tag="gg", name="gg")
            nc.gpsimd.dma_gather(
                out_ap=g[:, :nch * D].rearrange("p (c d) -> p c d", c=nch),
                in_ap=src_tables[s],
                idxs_ap=idx_t[:, :gi // 16],
                num_idxs=gi,
                num_idxs_reg=gi,
                elem_size=D,
                single_packet=False,
                queue_num=cx.qrr % 4,
            )
            cx.qrr += 1
        gv = cx.gvp.tile([P, (GI_MAX // P) * D], BF, tag="gv", name="gv")
        mul_eng = nc.vector
        mul_eng.tensor_mul(
            gv[:, :nch * D].rearrange("p (c d) -> p c d", c=nch),
            g[:, :nch * D].rearrange("p (c d) -> p c d", c=nch),
            val_sb[:, q0:q0 + nch].to_broadcast([P, nch, D]),
        )
        sel = {0: None, 1: None}
        selbase = {0: 0, 1: 0}
        if sel_dram is not None:
            # L1: DMA host-built one-hot matrices (packed per half)
            for half, hmask in ((0, hasA), (1, hasB)):
                pos = sel_pos[half]
                qs = [q for q in range(q0, q0 + nch) if hmask[q]]
                if not qs:
                    continue
                a0, a1 = int(pos[qs[0]]), int(pos[qs[-1]]) + 1
                assert a1 - a0 == len(qs)
                sel[half] = cx.selp.tile([P, (GI_MAX // P) * P], BF,
                                         tag=f"sel{half}", name=f"sel{half}")
                nc.sync.dma_start(out=sel[half][:, :(a1 - a0) * P],
                                  in_=sel_dram[half][:, a0 * P:a1 * P])
                selbase[half] = a0
        else:
            for half, hmask, lr_sb in ((0, hasA, lrA_sb), (1, hasB, lrB_sb)):
                for (ra, rb) in _runs(hmask[q0:q0 + nch]):
                    if sel[half] is None:
                        sel[half] = cx.selp.tile([P, (GI_MAX // P) * P], BF,
                                                 tag=f"sel{half}", name=f"sel{half}")
                    ln = rb - ra
                    cx.seli += 1
                    nc.vector.tensor_tensor(
                        out=sel[half][:, ra * P:rb * P].rearrange("p (c j) -> p c j", c=ln),
                        in0=cx.iota_bf[:].rearrange("p (o j) -> p o j", o=1).to_broadcast([P, ln, P]),
                        in1=lr_sb[:, q0 + ra:q0 + rb].to_broadcast([P, ln, P]),
                        op=ALU.is_equal)
        for k in range(nch):
            q = q0 + k
            w = int(ch_w[q])
            for half, hm, stX, spX, adX in ((0, hasA, stA, spA, addA),
                                            (1, hasB, stB, spB, addB)):
                if not hm[q]:
                    continue
                wf = 2 * w + half
                if stX[q]:
                    open_ps[wf] = cx.psp.tile(
                        [P, D], F32, space="PSUM",
                        tag=f"pseg{half}", name=f"pseg{half}", bufs=2)
                pt = open_ps[wf]
                kk = (int(sel_pos[half][q]) - selbase[half]) if sel_dram is not None else k
                nc.tensor.matmul(out=pt[:],
                                 lhsT=sel[half][:, kk * P:(kk + 1) * P],
                                 rhs=gv[:, k * D:(k + 1) * D],
                                 start=bool(stX[q]), stop=bool(spX[q]))
                if spX[q]:
                    t, sl = raw_put(wf)
                    if adX[q]:
                        nc.vector.tensor_add(t[:, sl], t[:, sl], pt[:])
                    else:
                        nc.scalar.activation(t[:, sl], pt[:], AF.Copy)
                    del open_ps[wf]
        if ag_after and bi in ag_after:
            for go in ag_after[bi]:
                go()
    assert not open_ps, f"{name}: unclosed psum groups {list(open_ps)}"


def emit_epilogue(cx, blocks, acc_sb, nprefix):
    """acc[:, w] += raw[:, w]/max(||raw_w||,1e-12) for windows 0..nprefix-1.
    blocks: list of (tile, nwin_in_tile)."""
    nc = cx.nc
    done = 0
    for (t, bw) in blocks:
        off = 0
        while off < bw and done < nprefix:
            ng = min(32, bw - off, nprefix - done)
            sl = slice(off * D, (off + ng) * D)
            sq = cx.ep.tile([P, 32 * D], F32, tag="ep_sq", name="ep_sq")
            nc.vector.tensor_mul(sq[:, :ng * D], t[:, sl], t[:, sl])
            ss = cx.ep.tile([P, 32], F32, tag="ep_ss", name="ep_ss")
            nc.vector.reduce_sum(ss[:, :ng], sq[:, :ng * D].rearrange("p (w d) -> p w d", w=ng),
                                 axis=mybir.AxisListType.X)
            snorm = cx.ep.tile([P, 32], F32, tag="ep_sn", name="ep_sn")
            nc.scalar.activation(snorm[:, :ng], ss[:, :ng], AF.Sqrt)
            nc.vector.tensor_scalar_max(snorm[:, :ng], snorm[:, :ng], 1e-12)
            rn = cx.ep.tile([P, 32], F32, tag="ep_rn", name="ep_rn")
            nc.vector.reciprocal(rn[:, :ng], snorm[:, :ng])
            contrib = cx.ep.tile([P, 32 * D], F32, tag="ep_ct", name="ep_ct")
            nc.vector.tensor_mul(
                contrib[:, :ng * D].rearrange("p (w d) -> p w d", w=ng),
                t[:, sl].rearrange("p (w d) -> p w d", w=ng),
                rn[:, :ng].to_broadcast([P, ng, D]),
            )
            nc.vector.tensor_add(acc_sb[:, done * D:(done + ng) * D],
                                 acc_sb[:, done * D:(done + ng) * D],
                                 contrib[:, :ng * D])
            done += ng
            off += ng


def indirect_gather_rows(cx, out_sb, table_ap, idx_sb, ncols):
    nc = cx.nc
    for k in range(ncols):
        nc.gpsimd.indirect_dma_start(
            out=out_sb[:, k * D:(k + 1) * D],
            out_offset=None,
            in_=table_ap,
            in_offset=bass.IndirectOffsetOnAxis(ap=idx_sb[:, k:k + 1], axis=0),
        )


def normalize_rows(cx, x_sb, ngroups, tag):
    nc = cx.nc
    sq = cx.lp.tile([P, ngroups * D], F32, tag=f"{tag}_sq")
    nc.vector.tensor_mul(sq[:], x_sb[:, :ngroups * D], x_sb[:, :ngroups * D])
    ss = cx.lp.tile([P, ngroups], F32, tag=f"{tag}_ss")
    nc.vector.reduce_sum(ss[:], sq[:].rearrange("p (w d) -> p w d", w=ngroups),
                         axis=mybir.AxisListType.X)
    sn = cx.lp.tile([P, ngroups], F32, tag=f"{tag}_sn")
    nc.scalar.activation(sn[:], ss[:], AF.Sqrt)
    nc.vector.tensor_scalar_max(sn[:], sn[:], 1e-12)
    rn = cx.lp.tile([P, ngroups], F32, tag=f"{tag}_rn")
    nc.vector.reciprocal(rn[:], sn[:])
    nc.vector.tensor_mul(
        x_sb[:, :ngroups * D].rearrange("p (w d) -> p w d", w=ngroups),
        x_sb[:, :ngroups * D].rearrange("p (w d) -> p w d", w=ngroups),
        rn[:].to_broadcast([P, ngroups, D]),
    )


def rowdot(cx, a_sb, b_sb, out_sb, ngroups, tag):
    nc = cx.nc
    t = cx.lp.tile([P, ngroups * D], F32, tag=f"{tag}_t")
    nc.vector.tensor_mul(t[:], a_sb[:, :ngroups * D], b_sb[:, :ngroups * D])
    nc.vector.reduce_sum(out_sb[:, :ngroups], t[:].rearrange("p (w d) -> p w d", w=ngroups),
                         axis=mybir.AxisListType.X)


def transpose_groups(cx, src_sb, ngroups, tag):
    nc = cx.nc
    out = cx.lp.tile([P, ngroups * P], F32, tag=f"{tag}_T")
    for g in range(ngroups):
        pt = cx.psp.tile([P, P], F32, space="PSUM", tag="tr_ps", bufs=1)
        nc.tensor.transpose(out=pt[:D, :P], in_=src_sb[:, g * D:(g + 1) * D],
                            identity=cx.ident[:])
        nc.vector.tensor_copy(out[:D, g * P:(g + 1) * P], pt[:D, :P])
    return out


def build(pp):
    dims = pp["dims"]
    R_acc1, R_f11 = dims["R_acc1"], dims["R_f11"]
    R_acc2, R_f12 = dims["R_acc2"], dims["R_f12"]
    RB = dims["RB"]
    blks_f11, blks_f12 = dims["blks_f11"], dims["blks_f12"]
    nsrc_ag = dims["nsrc_ag"]
    st = pp["streams"]

    nc = bacc.Bacc("TRN2", target_bir_lowering=False, debug=False,
                   num_devices=NCORES, num_swdge_queues=4)
    cx = Ctx()
    cx.nc = nc
    cx.qrr = 0
    cx.seli = 0

    # ---- dram inputs
    f0_acc_il_t = nc.dram_tensor("f0_acc_il", [R_acc1, D], F32, kind="ExternalInput")
    f0_acc_bl_t = nc.dram_tensor("f0_acc_bl", [R_acc2, D], F32, kind="ExternalInput")
    g_in = {}
    for nm in ("il1", "bl1", "il2", "bl2", "ag"):
        s = st[nm]
        tch = s["tch"]
        d = dict(
            lrA=nc.dram_tensor(f"{nm}_lrA", [128, tch], BF, kind="ExternalInput"),
            lrB=nc.dram_tensor(f"{nm}_lrB", [128, tch], BF, kind="ExternalInput"),
            val=nc.dram_tensor(f"{nm}_val", [128, tch],
                               BF if nm in ("il1", "bl1") else F32,
                               kind="ExternalInput"),
        )
        if nm in ("il1", "bl1"):
            d["pay"] = nc.dram_tensor(f"{nm}_pay", [128, tch * D], BF, kind="ExternalInput")
            for half in (0, 1):
                nq = s["selpack"][half][2]
                d[f"sel{half}"] = nc.dram_tensor(f"{nm}_sel{half}", [128, max(nq, 1) * P],
                                                 BF, kind="ExternalInput")
        else:
            d["idx"] = nc.dram_tensor(f"{nm}_idx", [128, tch * 8], I16, kind="ExternalInput")
        g_in[nm] = d
    debug = bool(int(os.environ.get("DSCBR_DEBUG", "0")))
    lidx = {k: nc.dram_tensor(f"loss_{k}", [128, v.shape[1]], I32, kind="ExternalInput")
            for k, v in pp["loss"][0].items()}
    aug_in = {k: nc.dram_tensor(k, [128, 16], I32, kind="ExternalInput")
              for k in ("aug_u_bl", "aug_b0_bl", "aug_b0_il")}
    out_t = nc.dram_tensor("out", [1, 2], F32, kind="ExternalOutput")

    with tile.TileContext(nc) as tc:
        cx.tc = tc
        es = []
        def pool(name, bufs, **kw):
            p = tc.tile_pool(name=name, bufs=bufs, **kw)
            es.append(p)
            return p.__enter__()
        cx.psp = pool("psum", 1, space="PSUM")
        cx.dramp = pool("dram", 1, space="DRAM")
        cx.cp = pool("const", 1)

        iota_i = cx.cp.tile([P, P], I32)
        nc.gpsimd.iota(iota_i[:], pattern=[[1, P]], base=0, channel_multiplier=0)
        cx.iota_bf = cx.cp.tile([P, P], BF)
        nc.vector.tensor_copy(cx.iota_bf[:], iota_i[:])
        cx.ident = cx.cp.tile([P, P], F32)
        make_identity(nc, cx.ident[:])
        ones_col = cx.cp.tile([P, 1], F32)
        nc.vector.memset(ones_col[:], 1.0)

        # ---------- scoped pools for the SpMM phases ----------
        es2 = []
        def pool2(name, bufs, **kw):
            p = tc.tile_pool(name=name, bufs=bufs, **kw)
            es2.append(p)
            return p.__enter__()
        cx.gsp = pool2("gstream", 6)
        cx.gp = pool2("gather", 8)
        cx.gvp = pool2("gval", 3)
        cx.idxp = pool2("gidx", 8)
        cx.selp = pool2("sel", 2)
        cx.ep = pool2("epil", 1)
        cx.mp = pool2("meta", 1)
        cx.accp = pool2("accs", 1)

        cx.meta = {}
        def load_meta(nm):
            s = st[nm]
            tch = s["tch"]
            vt = BF if nm in ("il1", "bl1") else F32
            lrA = cx.mp.tile([128, tch], BF, tag=f"{nm}_lrA", name=f"{nm}_lrA")
            lrB = cx.mp.tile([128, tch], BF, tag=f"{nm}_lrB", name=f"{nm}_lrB")
            vv = cx.mp.tile([128, tch], vt, tag=f"{nm}_vv", name=f"{nm}_vv")
            nc.sync.dma_start(out=lrA[:], in_=g_in[nm]["lrA"][:])
            nc.sync.dma_start(out=lrB[:], in_=g_in[nm]["lrB"][:])
            nc.sync.dma_start(out=vv[:], in_=g_in[nm]["val"][:])
            cx.meta[nm] = (lrA, lrB, vv)

        # acc buffers
        nacc1, nacc2 = R_acc1 // P, R_acc2 // P
        acc_il = cx.accp.tile([P, nacc1 * D], F32, tag="acc_il", name="acc_il")
        nc.sync.dma_start(out=acc_il[:].rearrange("p (w d) -> p w d", w=nacc1),
                          in_=f0_acc_il_t[:].rearrange("(w p) d -> p w d", p=P))
        acc_bl = cx.accp.tile([P, nacc2 * D], F32, tag="acc_bl", name="acc_bl")
        nc.sync.dma_start(out=acc_bl[:].rearrange("p (w d) -> p w d", w=nacc2),
                          in_=f0_acc_bl_t[:].rearrange("(w p) d -> p w d", p=P))

        # raw block tiles (32 windows each), shared by il1/bl1
        nblk = max(len(blks_f11), len(blks_f12))
        def raw_blocks(blks):
            tiles = []
            for i, bs in enumerate(blks):
                t = cx.accp.tile([P, 32 * D], F32, tag=f"rawblk{i}", name=f"rawblk{i}")
                tiles.append((t, bs // P))
            return tiles
        def raw_put_blocks(tiles):
            def put(wf):
                return tiles[wf // 32][0], slice((wf % 32) * D, (wf % 32 + 1) * D)
            return put

        # collective helper
        def ag_pair(nm, rows_in, rows_out):
            ain = cx.dramp.tile([rows_in, D], F32, tag=f"{nm}_agin", name=f"{nm}_agin")
            aout = cx.dramp.tile([rows_out, D], F32, addr_space="Shared",
                                 tag=f"{nm}_agout", name=f"{nm}_agout")
            return ain, aout

        def emit_l1(nm, blks, pay_t, last_batch_of_blk):
            tiles = raw_blocks(blks)
            outs = []
            ag_after = {}
            for i, bs in enumerate(blks):
                ain, aout = ag_pair(f"{nm}b{i}", bs, bs * NCORES)
                outs.append(aout)
                def mk(i=i, bs=bs, ain=ain, aout=aout):
                    def go():
                        t, nw = tiles[i]
                        nc.sync.dma_start(
                            out=ain[:].rearrange("(w p) d -> p w d", p=P),
                            in_=t[:, :nw * D].rearrange("p (w d) -> p w d", w=nw))
                        nc.gpsimd.collective_compute(
                            "AllGather", ALU.bypass,
                            replica_groups=[list(range(NCORES))],
                            ins=[ain[:].opt()], outs=[aout[:].opt()])
                    return go
                ag_after.setdefault(last_batch_of_blk[i], []).append(mk())
            packs = st[nm]["selpack"]
            emit_graph(cx, nm, st[nm], None, pay_t, None,
                       raw_put_blocks(tiles), ag_after=ag_after,
                       sel_dram={h: g_in[nm][f"sel{h}"] for h in (0, 1)},
                       sel_pos={h: packs[h][1] for h in (0, 1)})
            return tiles, outs

        def last_batches(s, blks):
            """batch index after which each block's drains are complete."""
            nbat = len(s["batches"])
            out = []
            for i in range(len(blks)):
                wlo, whi = (sum(b // P for b in blks[:i]),
                            sum(b // P for b in blks[:i + 1]))
                lb = 0
                for bi, (sg, q0, nch) in enumerate(s["batches"]):
                    for q in range(q0, q0 + nch):
                        w = int(s["ch_w"][q])
                        if wlo <= 2 * w < whi or wlo <= 2 * w + 1 < whi:
                            if s["wp_last_q"][w] == q:
                                lb = bi
                out.append(lb)
            return out

        # wp_last_q helper array on streams
        for nm in ("il1", "bl1", "il2", "bl2", "ag"):
            s = st[nm]
            wpl = np.full(s["nwp"], -1, np.int64)
            for w in range(s["nwp"]):
                wpl[w] = max(s["lastq"][2 * w], s["lastq"][2 * w + 1])
            s["wp_last_q"] = wpl

        # ---------------- il1 ----------------
        for nm in ("il1", "bl1", "il2", "bl2", "ag"):
            load_meta(nm)
        il1_tiles, f1_il_blks = emit_l1("il1", blks_f11, g_in["il1"]["pay"],
                                        last_batches(st["il1"], blks_f11))
        if debug:
            raw_dump = nc.dram_tensor("dbg_raw_il1b0", [blks_f11[0], D], F32,
                                      kind="ExternalOutput")
            t0, nw0 = il1_tiles[0]
            nc.sync.dma_start(out=raw_dump[:].rearrange("(w p) d -> p w d", p=P),
                              in_=t0[:, :nw0 * D].rearrange("p (w d) -> p w d", w=nw0))
        emit_epilogue(cx, il1_tiles, acc_il, nacc1)

        # ---------------- bl1 ----------------
        bl1_tiles, f1_bl_blks = emit_l1("bl1", blks_f12, g_in["bl1"]["pay"],
                                        last_batches(st["bl1"], blks_f12))
        emit_epilogue(cx, bl1_tiles, acc_bl, nacc2)

        # ---------------- il2 ----------------
        raw2 = cx.accp.tile([P, nacc1 * D], F32, tag="raw2", name="raw2")
        def raw2_put(wf):
            return raw2, slice(wf * D, (wf + 1) * D)
        emit_graph(cx, "il2", st["il2"], [t[:] for t in f1_il_blks], None,
                   g_in["il2"]["idx"], raw2_put)
        emit_epilogue(cx, [(raw2, nacc1)], acc_il, nacc1)
        acc_il_in, acc_il_full = ag_pair("accil", R_acc1, R_acc1 * NCORES)
        nc.sync.dma_start(out=acc_il_in[:].rearrange("(w p) d -> p w d", p=P),
                          in_=acc_il[:].rearrange("p (w d) -> p w d", w=nacc1))
        nc.gpsimd.collective_compute(
            "AllGather", ALU.bypass, replica_groups=[list(range(NCORES))],
            ins=[acc_il_in[:].opt()], outs=[acc_il_full[:].opt()])

        # ---------------- bl2 ----------------
        raw2b = cx.accp.tile([P, nacc2 * D], F32, tag="raw2b", name="raw2b")
        def raw2b_put(wf):
            return raw2b, slice(wf * D, (wf + 1) * D)
        emit_graph(cx, "bl2", st["bl2"], [t[:] for t in f1_bl_blks], None,
                   g_in["bl2"]["idx"], raw2b_put)
        emit_epilogue(cx, [(raw2b, nacc2)], acc_bl, nacc2)
        acc_bl_in, acc_bl_full = ag_pair("accbl", R_acc2, R_acc2 * NCORES)
        nc.sync.dma_start(out=acc_bl_in[:].rearrange("(w p) d -> p w d", p=P),
                          in_=acc_bl[:].rearrange("p (w d) -> p w d", w=nacc2))
        nc.gpsimd.collective_compute(
            "AllGather", ALU.bypass, replica_groups=[list(range(NCORES))],
            ins=[acc_bl_in[:].opt()], outs=[acc_bl_full[:].opt()])

        # ---------------- agg ----------------
        nwB = RB // P
        rawag = cx.accp.tile([P, nwB * D], F32, tag="rawag", name="rawag")
        def rawag_put(wf):
            return rawag, slice(wf * D, (wf + 1) * D)
        acc_il_ap = acc_il_full[:]
        ag_tables = [acc_il_ap[s * SRC_WIN: min((s + 1) * SRC_WIN, R_acc1 * NCORES), :]
                     for s in range(nsrc_ag)]
        emit_graph(cx, "ag", st["ag"], ag_tables, None, g_in["ag"]["idx"], rawag_put)
        ilb_in, ilb_full = ag_pair("ilb", RB, RB * NCORES)
        nc.sync.dma_start(out=ilb_in[:].rearrange("(w p) d -> p w d", p=P),
                          in_=rawag[:, :nwB * D].rearrange("p (w d) -> p w d", w=nwB))
        nc.gpsimd.collective_compute(
            "AllGather", ALU.bypass, replica_groups=[list(range(NCORES))],
            ins=[ilb_in[:].opt()], outs=[ilb_full[:].opt()])

        if debug:
            for nm, t, rows in (("dbg_acc_il", acc_il_full, R_acc1 * NCORES),
                                ("dbg_acc_bl", acc_bl_full, R_acc2 * NCORES),
                                ("dbg_ilb", ilb_full, RB * NCORES),
                                ("dbg_f1il0", f1_il_blks[0], blks_f11[0] * NCORES)):
                o = nc.dram_tensor(nm, [rows, D], F32, kind="ExternalOutput")
                nc.sync.dma_start(out=o[:], in_=t[:])

        for p in reversed(es2):
            p.__exit__(None, None, None)
        cx.lp = pool("loss", 1)

        # ---------------- loss ----------------
        bsh = BATCH // NCORES          # 256
        ng = bsh // P                  # 2
        lidx_sb = {}
        for k, t in lidx.items():
            s = cx.lp.tile([128, t.shape[1]], I32, tag=f"li_{k}")
            nc.sync.dma_start(out=s[:], in_=t[:])
            lidx_sb[k] = s
        for k, t in aug_in.items():
            s = cx.lp.tile([128, 16], I32, tag=f"li_{k}")
            nc.sync.dma_start(out=s[:], in_=t[:])
            lidx_sb[k] = s

        def gather(tag, table, idxk, ncols):
            sb = cx.lp.tile([P, ncols * D], F32, tag=tag)
            indirect_gather_rows(cx, sb, table, lidx_sb[idxk], ncols)
            return sb
        pos_u_il = gather("pos_u_il", acc_il_full[:], "u_il", ng)
        pos_u_bl = gather("pos_u_bl", acc_bl_full[:], "u_bl", ng)
        b_il0 = gather("b_il0", ilb_full[:], "b_il0", ng)
        b_il1 = gather("b_il1", ilb_full[:], "b_il1", ng)
        b_bl0 = gather("b_bl0", acc_bl_full[:], "b_bl0", ng)
        b_bl1 = gather("b_bl1", acc_bl_full[:], "b_bl1", ng)
        aug_u = gather("aug_u", acc_bl_full[:], "aug_u_bl", 16)
        aug_b = gather("aug_b", acc_bl_full[:], "aug_b0_bl", 16)
        # -- bpr
        pr0 = cx.lp.tile([P, ng], F32, tag="pr0")
        pr1 = cx.lp.tile([P, ng], F32, tag="pr1")
        tmp = cx.lp.tile([P, ng], F32, tag="prt")
        rowdot(cx, pos_u_il, b_il0, pr0, ng, "d0")
        rowdot(cx, pos_u_bl, b_bl0, tmp, ng, "d1")
        nc.vector.tensor_add(pr0[:], pr0[:], tmp[:])
        rowdot(cx, pos_u_il, b_il1, pr1, ng, "d2")
        rowdot(cx, pos_u_bl, b_bl1, tmp, ng, "d3")
        nc.vector.tensor_add(pr1[:], pr1[:], tmp[:])
        x = cx.lp.tile([P, ng], F32, tag="bprx")
        nc.vector.tensor_tensor(out=x[:], in0=pr1[:], in1=pr0[:], op=ALU.subtract)
        negx = cx.lp.tile([P, ng], F32, tag="bprnx")
        nc.vector.tensor_scalar_mul(negx[:], x[:], -1.0)
        nax = cx.lp.tile([P, ng], F32, tag="bprax")
        nc.vector.tensor_tensor(out=nax[:], in0=x[:], in1=negx[:], op=ALU.min)
        e = cx.lp.tile([P, ng], F32, tag="bpre")
        nc.scalar.activation(e[:], nax[:], AF.Exp)
        nc.vector.tensor_scalar_add(e[:], e[:], 1.0)
        l1p = cx.lp.tile([P, ng], F32, tag="bprl")
        nc.scalar.activation(l1p[:], e[:], AF.Ln)
        sp = cx.lp.tile([P, ng], F32, tag="bprsp")
        nc.vector.tensor_scalar_max(sp[:], x[:], 0.0)
        nc.vector.tensor_add(sp[:], sp[:], l1p[:])

        normalize_rows(cx, aug_u, 16, "nau")
        normalize_rows(cx, aug_b, 16, "nab")
        normalize_rows(cx, pos_u_il, ng, "npu")
        my_pos_b = cx.lp.tile([P, ng * D], F32, tag="my_pb")
        nc.vector.tensor_copy(my_pos_b[:], b_il0[:, :ng * D])
        normalize_rows(cx, my_pos_b, ng, "npb")

        part = cx.lp.tile([P, 4], F32, tag="parts")
        nc.vector.memset(part[:], 0.0)
        nc.vector.reduce_sum(part[:, 0:1], sp[:].rearrange("p (w d) -> p w d", w=1),
                             axis=mybir.AxisListType.X)

        def closs_partial(pos_my, aug_full, aug_my_cols, out_col):
            posT = transpose_groups(cx, pos_my, ng, "pT")
            augT = transpose_groups(cx, aug_full, 16, "aT")
            ps = cx.lp.tile([P, ng], F32, tag="ps")
            rowdot(cx, pos_my, aug_my_cols, ps, ng, f"psd{out_col}")
            lse = cx.lp.tile([P, ng], F32, tag="lse")
            for g in range(ng):
                ttl_ps = cx.psp.tile([P, 512], F32, space="PSUM", tag="ttl", bufs=1)
                ttl = cx.lp.tile([P, BATCH], F32, tag="ttl")
                for nb_ in range(BATCH // 512):
                    nc.tensor.matmul(
                        out=ttl_ps[:, :512],
                        lhsT=posT[:D, g * P:(g + 1) * P],
                        rhs=augT[:D, nb_ * 512:(nb_ + 1) * 512],
                        start=True, stop=True)
                    nc.vector.tensor_copy(ttl[:, nb_ * 512:(nb_ + 1) * 512], ttl_ps[:, :512])
                mx = cx.lp.tile([P, 1], F32, tag="mx")
                nc.vector.reduce_max(mx[:], ttl[:].rearrange("p (w d) -> p w d", w=1),
                                     axis=mybir.AxisListType.X)
                nmx = cx.lp.tile([P, 1], F32, tag="nmx")
                nc.vector.tensor_scalar_mul(nmx[:], mx[:], -4.0)
                ex = cx.lp.tile([P, BATCH], F32, tag="ex")
                se = cx.lp.tile([P, 1], F32, tag="se")
                nc.scalar.activation(ex[:], ttl[:], AF.Exp, bias=nmx[:, :1], scale=4.0,
                                     accum_out=se[:, :1])
                ln = cx.lp.tile([P, 1], F32, tag="ln")
                nc.scalar.activation(ln[:], se[:], AF.Ln)
                m4 = cx.lp.tile([P, 1], F32, tag="m4")
                nc.vector.tensor_scalar_mul(m4[:], mx[:], 4.0)
                nc.vector.tensor_add(lse[:, g:g + 1], ln[:], m4[:])
            t4 = cx.lp.tile([P, ng], F32, tag="t4")
            nc.vector.tensor_scalar_mul(t4[:], ps[:], 4.0)
            nc.vector.tensor_tensor(out=t4[:], in0=t4[:], in1=lse[:], op=ALU.subtract)
            nc.vector.reduce_sum(part[:, out_col:out_col + 1],
                                 t4[:].rearrange("p (w d) -> p w d", w=1),
                                 axis=mybir.AxisListType.X)

        aug_u_my = gather("aug_u_my", acc_bl_full[:], "u_bl", ng)
        normalize_rows(cx, aug_u_my, ng, "naum")
        aug_b_my = gather("aug_b_my", acc_bl_full[:], "b_bl0", ng)
        normalize_rows(cx, aug_b_my, ng, "nabm")
        closs_partial(pos_u_il, aug_u, aug_u_my, 1)
        closs_partial(my_pos_b, aug_b, aug_b_my, 2)

        pp_ps = cx.psp.tile([P, 4], F32, space="PSUM", tag="ppps", bufs=1)
        nc.tensor.matmul(out=pp_ps[:1, :4], lhsT=ones_col[:], rhs=part[:],
                         start=True, stop=True)
        psum_sb = cx.lp.tile([1, 4], F32, tag="psums")
        nc.vector.tensor_copy(psum_sb[:], pp_ps[:1, :4])
        ar_in = cx.dramp.tile([1, 4], F32, tag="ar_in")
        ar_out = cx.dramp.tile([1, 4], F32, addr_space="Shared", tag="ar_out")
        nc.sync.dma_start(out=ar_in[:], in_=psum_sb[:])
        nc.gpsimd.collective_compute(
            "AllReduce", ALU.add, replica_groups=[list(range(NCORES))],
            ins=[ar_in[:].opt()], outs=[ar_out[:].opt()])
        fin = cx.lp.tile([1, 4], F32, tag="fin")
        nc.sync.dma_start(out=fin[:], in_=ar_out[:])
        res = cx.lp.tile([1, 2], F32, tag="res")
        nc.vector.tensor_scalar_mul(res[:, 0:1], fin[:, 0:1], 1.0 / BATCH)
        t = cx.lp.tile([1, 1], F32, tag="rt")
        nc.vector.tensor_add(t[:], fin[:, 1:2], fin[:, 2:3])
        nc.vector.tensor_scalar_mul(res[:, 1:2], t[:], -0.5 / BATCH)
        nc.sync.dma_start(out=out_t[:], in_=res[:])

        for p in reversed(es):
            p.__exit__(None, None, None)
    nc.compile()
    return nc


# ---------------------------------------------------------------- entry point

def _install_ntff_hook():
    if "antenv.axon_hooks" in sys.modules:
        return
    try:
        mod = types.ModuleType("antenv.axon_hooks")
        _hook = [None]
        mod.set_axon_ntff_profile_hook = lambda h: _hook.__setitem__(0, h)
        mod.get_axon_ntff_profile_hook = lambda: _hook[0]
        sys.modules["antenv.axon_hooks"] = mod
        import antenv
        antenv.axon_hooks = mod
        from trn_agent_boot.trn_boot import _ntff_profile_via_ctypes
        hook = _ntff_profile_via_ctypes("/opt/axon/libaxon_pjrt.so")
        if hook is not None:
            mod.set_axon_ntff_profile_hook(hook)
    except Exception:
        pass


def make_in_maps(pp):
    import ml_dtypes
    maps = []
    st = pp["streams"]
    for c in range(NCORES):
        m = {
            "f0_acc_il": pp["f0_acc"]["il"][c],
            "f0_acc_bl": pp["f0_acc"]["bl"][c],
            "aug_u_bl": pp["aug_u_bl"], "aug_b0_bl": pp["aug_b0_bl"],
            "aug_b0_il": pp["aug_b0_il"],
        }
        for nm in ("il1", "bl1", "il2", "bl2", "ag"):
            s = st[nm]
            m[f"{nm}_lrA"] = np.ascontiguousarray(
                s["lrA"][c].reshape(-1, P).T).astype(ml_dtypes.bfloat16)
            m[f"{nm}_lrB"] = np.ascontiguousarray(
                s["lrB"][c].reshape(-1, P).T).astype(ml_dtypes.bfloat16)
            vals = np.ascontiguousarray(s["val"][c].reshape(-1, P).T)
            if nm in ("il1", "bl1"):
                m[f"{nm}_val"] = vals.astype(ml_dtypes.bfloat16)
                m[f"{nm}_pay"] = pp["pay"][nm][c]
                for h in (0, 1):
                    mats, pos, nq = s["selpack"][h]
                    m[f"{nm}_sel{h}"] = mats[c] if nq else np.zeros(
                        (128, P), ml_dtypes.bfloat16)
            else:
                m[f"{nm}_val"] = vals
                m[f"{nm}_idx"] = wrap_idx16(s["srcv"][c].astype(np.int16))
        for k, v in pp["loss"][c].items():
            m[f"loss_{k}"] = v
        maps.append(m)
    return maps


_CACHE = {}


def kernel(**inputs) -> np.ndarray:
    _install_ntff_hook()
    pp = preprocess(inputs)
    key = tuple(sorted((k, v) for k, v in pp["dims"].items()
                       if isinstance(v, int)))
    if key not in _CACHE:
        _CACHE[key] = build(pp)
    nc = _CACHE[key]
    in_maps = make_in_maps(pp)
    trace = bool(int(os.environ.get("DSCBR_TRACE", "0")))
    res = run_bass_kernel_spmd(nc, in_maps, core_ids=list(range(NCORES)), trace=trace)
    if trace and res.exec_time_ns:
        print(f"HW exec time: {res.exec_time_ns} ns")
    kernel._last_results = res.results
    out = res.results[0]["out"].reshape(2).astype(np.float32)
    return out


# revision 49
# speedup vs baseline: 1.0421x; 1.0421x over previous
"""Trainium2 Bass kernel for nn_DSCBR (gnn_message_passing).

Strategy (8 NeuronCores, SPMD, dest-sharded):
- Host prunes both propagation graphs by backward slicing from the loss batch
  (only rows that feed the final losses are computed), then compacts each
  layer's destination space; compact rows are round-robin sharded.
- Layer-1 SpMM sources come from the INPUT feature tables, so the host
  pre-gathers them into per-core edge-ordered payload streams (pure indexing;
  all FP math stays on device).  The device streams payloads contiguously
  (HWDGE), multiplies by edge values, and segment-sums via selection-matrix
  matmuls accumulated in PSUM bank tiles.
- Layer-2/agg SpMM sources are runtime tables; gathered per edge with
  dma_gather spread over 4 SWDGE queues.
- f1 tables are all-gathered in per-block sub-collectives so layer-2 can
  start on block 0 while later blocks are still in flight.
- Losses (BPR + two contrastive views) computed batch-sharded + AllReduce.
"""
import os
import sys
import types

sys.path.insert(0, "/opt/trn_rl_repo")

import numpy as np

import concourse.bass as bass
import concourse.bacc as bacc
import concourse.mybir as mybir
import concourse.tile as tile
from concourse.bass_utils import run_bass_kernel_spmd
from concourse.masks import make_identity

P = 128
NCORES = 8
SRC_WIN = 32768
BLK_SLOTS = SRC_WIN // NCORES   # 4096 per-core slots per AG block
GI_MAX = 2048
D = 64
NU, NI, NB = 100000, 50000, 20000
BATCH = 2048
F32 = mybir.dt.float32
I32 = mybir.dt.int32
I16 = mybir.dt.int16
BF = mybir.dt.bfloat16
AF = mybir.ActivationFunctionType
ALU = mybir.AluOpType


# ---------------------------------------------------------------- host prep

def _pad_ids(real, n_space, mult):
    """real: sorted unique ids. Append complement ids to a multiple of mult."""
    need = (-len(real)) % mult
    if need == 0:
        return np.asarray(real, np.int64)
    m = np.ones(n_space, bool)
    m[real] = False
    pad = np.flatnonzero(m)[:need]
    assert len(pad) == need, "no room to pad id set"
    return np.concatenate([np.asarray(real, np.int64), pad])


def _pad_concat(base, extra, n_space, mult):
    arr = np.concatenate([np.asarray(base, np.int64), np.asarray(extra, np.int64)])
    need = (-len(arr)) % mult
    if need == 0:
        return arr
    m = np.ones(n_space, bool)
    m[arr] = False
    pad = np.flatnonzero(m)[:need]
    assert len(pad) == need
    return np.concatenate([arr, pad])


def _posmap(ids, n_space):
    g = np.full(n_space, -1, np.int64)
    g[ids] = np.arange(len(ids))
    return g


def _blk_sizes(R):
    """Per-core block slot counts (multiples of 128), blocks of <=BLK_SLOTS."""
    out = []
    left = R
    while left > 0:
        out.append(min(BLK_SLOTS, left))
        left -= out[-1]
    return out


def _runs(mask):
    """Maximal [a,b) runs of True in a 1-d bool array."""
    out = []
    a = None
    for i, v in enumerate(mask):
        if v and a is None:
            a = i
        elif not v and a is not None:
            out.append((a, i))
            a = None
    if a is not None:
        out.append((a, len(mask)))
    return out


def build_stream(rows_pos, swin, srcv, vals, R, nsrc, ncores=NCORES):
    """Build per-core dest-sharded edge streams with (swin, win-pair, lrow)
    grouping, 128-edge chunks padded to the max count over cores.

    Returns a dict with per-core streams and the shared chunk program."""
    nwp = R // (2 * P)
    rows_pos = np.asarray(rows_pos, np.int64)
    swin = np.asarray(swin, np.int64)
    core = rows_pos % ncores
    slot = rows_pos // ncores
    wp = slot // (2 * P)
    lrow = slot % (2 * P)
    order = np.lexsort((lrow, wp, swin, core))
    c_s, s_s, w_s, l_s = core[order], swin[order], wp[order], lrow[order]
    sv_s, v_s = np.asarray(srcv)[order], np.asarray(vals)[order]

    counts = np.zeros((ncores, nsrc, nwp), np.int64)
    np.add.at(counts, (c_s, s_s, w_s), 1)
    countsA = np.zeros((ncores, nsrc, nwp), np.int64)
    mA = l_s < P
    np.add.at(countsA, (c_s[mA], s_s[mA], w_s[mA]), 1)
    maxc = counts.max(axis=0)
    nchunks = (maxc + P - 1) // P                     # [nsrc, nwp]
    tch = int(nchunks.sum())
    grp_choff = np.concatenate([[0], np.cumsum(nchunks.ravel())])[:-1].reshape(nsrc, nwp)

    key = (c_s * nsrc + s_s) * nwp + w_s
    starts = np.searchsorted(key, np.arange(ncores * nsrc * nwp))
    ends = np.searchsorted(key, np.arange(ncores * nsrc * nwp) + 1)

    srcv_st = np.zeros((ncores, tch * P), np.int64)
    lrA = np.full((ncores, tch * P), 300.0, np.float32)
    lrB = np.full((ncores, tch * P), 300.0, np.float32)
    val_st = np.zeros((ncores, tch * P), np.float32)
    ch_s = np.zeros(tch, np.int64)
    ch_w = np.zeros(tch, np.int64)
    hasA = np.zeros(tch, bool)
    hasB = np.zeros(tch, bool)
    for s in range(nsrc):
        for w in range(nwp):
            ncw = int(nchunks[s, w])
            if ncw == 0:
                continue
            off = int(grp_choff[s, w])
            ch_s[off:off + ncw] = s
            ch_w[off:off + ncw] = w
            for c in range(ncores):
                k = (c * nsrc + s) * nwp + w
                a, b = int(starts[k]), int(ends[k])
                n = b - a
                if n == 0:
                    continue
                pos = off * P
                srcv_st[c, pos:pos + n] = sv_s[a:b]
                lr = l_s[a:b]
                lrA[c, pos:pos + n] = np.where(lr < P, lr, 300.0)
                lrB[c, pos:pos + n] = np.where(lr >= P, lr - P, 300.0)
                val_st[c, pos:pos + n] = v_s[a:b]
                nAc = int(countsA[c, s, w])
                kA = (nAc + P - 1) // P
                hasA[off:off + kA] = True
                if n > nAc:
                    hasB[off + nAc // P: off + (n + P - 1) // P] = True

    nwin = R // P
    firstq = np.full(2 * nwp, -1, np.int64)
    lastq = np.full(2 * nwp, -1, np.int64)
    for q in range(tch):
        for half, has in ((0, hasA[q]), (1, hasB[q])):
            if has:
                wf = 2 * ch_w[q] + half
                if firstq[wf] < 0:
                    firstq[wf] = q
                lastq[wf] = q
    memset_wf = [wf for wf in range(nwin) if firstq[wf] < 0]

    # per-(segment, wp, half) start/stop flags and drain ops.
    # PSUM accumulation groups must not interleave within a bank, so each
    # (s, w, half) is its own group in its own tile; drains copy on the
    # half's first touched segment and add on later ones.
    stA = np.zeros(tch, bool)
    spA = np.zeros(tch, bool)
    stB = np.zeros(tch, bool)
    spB = np.zeros(tch, bool)
    addA = np.zeros(tch, bool)   # at spA chunk: accumulate into raw, not copy
    addB = np.zeros(tch, bool)
    seen_wf = set()
    for s in range(nsrc):
        for w in range(nwp):
            ncw = int(nchunks[s, w])
            if ncw == 0:
                continue
            off = int(grp_choff[s, w])
            qs = np.arange(off, off + ncw)
            for half, hm, stX, spX, adX in ((0, hasA, stA, spA, addA),
                                            (1, hasB, stB, spB, addB)):
                qa = qs[hm[qs]]
                if len(qa) == 0:
                    continue
                stX[qa[0]] = True
                spX[qa[-1]] = True
                wf = 2 * w + half
                if wf in seen_wf:
                    adX[qa[-1]] = True
                seen_wf.add(wf)

    batches = []
    for s in range(nsrc):
        lo = int(grp_choff[s, 0])
        hi = int(grp_choff[s, nwp - 1] + nchunks[s, nwp - 1]) if nwp else lo
        q0 = lo
        while q0 < hi:
            nch = min(GI_MAX // P, hi - q0)
            batches.append((s, q0, nch))
            q0 += nch

    return dict(srcv=srcv_st, lrA=lrA, lrB=lrB, val=val_st,
                ch_s=ch_s, ch_w=ch_w, hasA=hasA, hasB=hasB,
                firstq=firstq, lastq=lastq, memset_wf=memset_wf,
                stA=stA, spA=spA, stB=stB, spB=spB, addA=addA, addB=addB,
                batches=batches, tch=tch, nwp=nwp, R=R, nsrc=nsrc)


def wrap_idx16(flat):
    # index i -> partition i%16, col i//16; replicated x8 down partitions
    return np.ascontiguousarray(np.tile(flat.reshape(-1, 16).T.astype(np.int16), (8, 1)))


def idx_cols_i32(flat):
    # [n] -> [128, n/128] int32; col k = rows [128k, 128k+128)
    n = flat.shape[0]
    assert n % P == 0
    return np.ascontiguousarray(flat.reshape(-1, P).T.astype(np.int32))


def _table_pos(g, R):
    """compact position -> row in the core-slab-major all-gathered table."""
    return (g % NCORES) * R + g // NCORES


def _blk_pos(g, sizes):
    """compact position -> (block, idx within block) for block-major tables."""
    c, s = g % NCORES, g // NCORES
    k = s // BLK_SLOTS
    sz = np.asarray(sizes, np.int64)[k]
    return k, c * sz + (s - k * BLK_SLOTS)


def preprocess(inputs, ncores=NCORES):
    u = np.asarray(inputs["users_feature"], np.float32)
    it = np.asarray(inputs["items_feature"], np.float32)
    b = np.asarray(inputs["bundles_feature"], np.float32)
    f0_il = np.concatenate([u, it], 0)
    f0_bl = np.concatenate([u, b], 0)
    N1, N2 = NU + NI, NU + NB

    il_row = np.asarray(inputs["il_row"], np.int64)
    il_col = np.asarray(inputs["il_col"], np.int64)
    il_val = np.asarray(inputs["il_val"], np.float32)
    bl_row = np.asarray(inputs["bl_row"], np.int64)
    bl_col = np.asarray(inputs["bl_col"], np.int64)
    bl_val = np.asarray(inputs["bl_val"], np.float32)
    agg_row = np.asarray(inputs["agg_row"], np.int64)
    agg_col = np.asarray(inputs["agg_col"], np.int64)
    agg_val = np.asarray(inputs["agg_val"], np.float32)
    users = np.asarray(inputs["users"], np.int64)
    bundles = np.asarray(inputs["bundles"], np.int64)

    # ---- active sets (backward slice from the loss batch)
    mB = np.zeros(NB, bool)
    mB[bundles.ravel()] = True
    BstarP = _pad_ids(np.flatnonzero(mB), NB, 2048)
    gB = _posmap(BstarP, NB)
    keep_ag = mB[agg_row]
    items = np.unique(agg_col[keep_ag])
    uuniq = np.unique(users)

    # il graph
    S_acc_il = np.union1d(uuniq, NU + items)
    S_acc_ilP = _pad_ids(S_acc_il, N1, 2048)
    g_acc_il = _posmap(S_acc_ilP, N1)
    acc_mask_il = np.zeros(N1, bool)
    acc_mask_il[S_acc_il] = True
    keep2_il = acc_mask_il[il_row]
    S2_il = np.unique(il_col[keep2_il])
    inP = np.zeros(N1, bool)
    inP[S_acc_ilP] = True
    F1_ilP = _pad_concat(S_acc_ilP, S2_il[~inP[S2_il]], N1, 2048)
    g_f1_il = _posmap(F1_ilP, N1)
    f1_mask_il = acc_mask_il.copy()
    f1_mask_il[S2_il] = True
    keep1_il = f1_mask_il[il_row]

    # bl graph
    S_acc_bl = np.union1d(uuniq, NU + BstarP)
    S_acc_blP = _pad_ids(S_acc_bl, N2, 2048)
    g_acc_bl = _posmap(S_acc_blP, N2)
    acc_mask_bl = np.zeros(N2, bool)
    acc_mask_bl[S_acc_bl] = True
    keep2_bl = acc_mask_bl[bl_row]
    S2_bl = np.unique(bl_col[keep2_bl])
    inP2 = np.zeros(N2, bool)
    inP2[S_acc_blP] = True
    F1_blP = _pad_concat(S_acc_blP, S2_bl[~inP2[S2_bl]], N2, 2048)
    g_f1_bl = _posmap(F1_blP, N2)
    f1_mask_bl = acc_mask_bl.copy()
    f1_mask_bl[S2_bl] = True
    keep1_bl = f1_mask_bl[bl_row]

    R_acc1 = len(S_acc_ilP) // ncores
    R_f11 = len(F1_ilP) // ncores
    R_acc2 = len(S_acc_blP) // ncores
    R_f12 = len(F1_blP) // ncores
    RB = len(BstarP) // ncores
    blks_f11 = _blk_sizes(R_f11)
    blks_f12 = _blk_sizes(R_f12)

    # ---- edge streams
    il1 = build_stream(g_f1_il[il_row[keep1_il]], np.zeros(int(keep1_il.sum()), np.int64),
                       il_col[keep1_il], il_val[keep1_il], R_f11, 1)
    bl1 = build_stream(g_f1_bl[bl_row[keep1_bl]], np.zeros(int(keep1_bl.sum()), np.int64),
                       bl_col[keep1_bl], bl_val[keep1_bl], R_f12, 1)

    p2 = g_f1_il[il_col[keep2_il]]
    k2, i2 = _blk_pos(p2, blks_f11)
    il2 = build_stream(g_acc_il[il_row[keep2_il]], k2, i2,
                       il_val[keep2_il], R_acc1, len(blks_f11))
    p2b = g_f1_bl[bl_col[keep2_bl]]
    k2b, i2b = _blk_pos(p2b, blks_f12)
    bl2 = build_stream(g_acc_bl[bl_row[keep2_bl]], k2b, i2b,
                       bl_val[keep2_bl], R_acc2, len(blks_f12))

    pag = _table_pos(g_acc_il[NU + agg_col[keep_ag]], R_acc1)
    nsrc_ag = (ncores * R_acc1 + SRC_WIN - 1) // SRC_WIN
    ag = build_stream(gB[agg_row[keep_ag]], pag // SRC_WIN, pag % SRC_WIN,
                      agg_val[keep_ag], RB, nsrc_ag)

    # ---- L1 payloads (host pre-gather of INPUT features; edge-ordered)
    import ml_dtypes
    def payload(st, f0):
        out = []
        for c in range(ncores):
            arr = f0[st["srcv"][c]]                     # [tch*128, D]
            arr = arr.reshape(st["tch"], P, D).transpose(1, 0, 2).reshape(P, st["tch"] * D)
            out.append(np.ascontiguousarray(arr.astype(ml_dtypes.bfloat16)))
        return out
    il1_pay = payload(il1, f0_il)
    bl1_pay = payload(bl1, f0_bl)

    # ---- initial acc rows (f0 at compact acc rows, per core)
    f0_acc_il = [np.ascontiguousarray(f0_il[S_acc_ilP[c::ncores]]) for c in range(ncores)]
    f0_acc_bl = [np.ascontiguousarray(f0_bl[S_acc_blP[c::ncores]]) for c in range(ncores)]

    # ---- loss indices
    loss = {}
    bsh = BATCH // ncores
    for c in range(ncores):
        sl = slice(c * bsh, (c + 1) * bsh)
        loss[c] = dict(
            u_il=idx_cols_i32(_table_pos(g_acc_il[users[sl]], R_acc1)),
            u_bl=idx_cols_i32(_table_pos(g_acc_bl[users[sl]], R_acc2)),
            b_il0=idx_cols_i32(_table_pos(gB[bundles[sl, 0]], RB)),
            b_il1=idx_cols_i32(_table_pos(gB[bundles[sl, 1]], RB)),
            b_bl0=idx_cols_i32(_table_pos(g_acc_bl[bundles[sl, 0] + NU], R_acc2)),
            b_bl1=idx_cols_i32(_table_pos(g_acc_bl[bundles[sl, 1] + NU], R_acc2)),
        )
    aug_u_bl = idx_cols_i32(_table_pos(g_acc_bl[users], R_acc2))
    aug_b0_bl = idx_cols_i32(_table_pos(g_acc_bl[bundles[:, 0] + NU], R_acc2))
    aug_b0_il = idx_cols_i32(_table_pos(gB[bundles[:, 0]], RB))

    return dict(streams=dict(il1=il1, bl1=bl1, il2=il2, bl2=bl2, ag=ag),
                pay=dict(il1=il1_pay, bl1=bl1_pay),
                f0_acc=dict(il=f0_acc_il, bl=f0_acc_bl),
                loss=loss, aug_u_bl=aug_u_bl, aug_b0_bl=aug_b0_bl,
                aug_b0_il=aug_b0_il,
                dims=dict(R_acc1=R_acc1, R_f11=R_f11, R_acc2=R_acc2,
                          R_f12=R_f12, RB=RB, blks_f11=blks_f11,
                          blks_f12=blks_f12, nsrc_ag=nsrc_ag))


# ---------------------------------------------------------------- bass build

class Ctx:
    pass


NPB = 5  # psum bank tiles (8 windows each)


def emit_graph(cx, name, st, src_tables, pay_dram, idx_dram, raw_put, ag_after=None):
    """Emit one SpMM layer.

    src_tables: list of table APs per source window (gather mode), or None.
    pay_dram: payload dram tensor (stream mode), or None.
    raw_put(wf) -> (tile, col_slice) drain destination.
    ag_after: optional dict {batch_index: callable} to emit sub-AG right
    after that batch (pipelined collectives).
    """
    nc = cx.nc
    lrA_sb, lrB_sb, val_sb = cx.meta[name]
    for wf in st["memset_wf"]:
        t, sl = raw_put(wf)
        nc.vector.memset(t[:, sl], 0.0)

    hasA, hasB = st["hasA"], st["hasB"]
    ch_w = st["ch_w"]
    stA, spA, stB, spB = st["stA"], st["spA"], st["stB"], st["spB"]
    addA, addB = st["addA"], st["addB"]
    open_ps = {}

    for bi, (s, q0, nch) in enumerate(st["batches"]):
        gi = nch * P
        if pay_dram is not None:
            g = cx.gsp.tile([P, (GI_MAX // P) * D], BF, tag="gs", name="gs")
            nc.sync.dma_start(out=g[:, :nch * D], in_=pay_dram[:, q0 * D:(q0 + nch) * D])
        else:
            idx_t = cx.idxp.tile([128, GI_MAX // 16], I16, tag="gidx", name="gidx")
            nc.sync.dma_start(out=idx_t[:, :gi // 16],
                              in_=idx_dram[:, q0 * 8:(q0 + nch) * 8])
            g = cx.gp.tile([P, (GI_MAX // P) * D], F32, tag="gg", name="gg")
            nc.gpsimd.dma_gather(
                out_ap=g[:, :nch * D].rearrange("p (c d) -> p c d", c=nch),
                in_ap=src_tables[s],
                idxs_ap=idx_t[:, :gi // 16],
                num_idxs=gi,
                num_idxs_reg=gi,
                elem_size=D,
                single_packet=False,
                queue_num=cx.qrr % 4,
            )
            cx.qrr += 1
        gv = cx.gvp.tile([P, (GI_MAX // P) * D], BF, tag="gv", name="gv")
        mul_eng = nc.vector
        mul_eng.tensor_mul(
            gv[:, :nch * D].rearrange("p (c d) -> p c d", c=nch),
            g[:, :nch * D].rearrange("p (c d) -> p c d", c=nch),
            val_sb[:, q0:q0 + nch].to_broadcast([P, nch, D]),
        )
        sel = {0: None, 1: None}
        for half, hmask, lr_sb in ((0, hasA, lrA_sb), (1, hasB, lrB_sb)):
            for (ra, rb) in _runs(hmask[q0:q0 + nch]):
                if sel[half] is None:
                    sel[half] = cx.selp.tile([P, (GI_MAX // P) * P], BF,
                                             tag=f"sel{half}", name=f"sel{half}")
                ln = rb - ra
                nc.vector.tensor_tensor(
                    out=sel[half][:, ra * P:rb * P].rearrange("p (c j) -> p c j", c=ln),
                    in0=cx.iota_bf[:].rearrange("p (o j) -> p o j", o=1).to_broadcast([P, ln, P]),
                    in1=lr_sb[:, q0 + ra:q0 + rb].to_broadcast([P, ln, P]),
                    op=ALU.is_equal)
        for k in range(nch):
            q = q0 + k
            w = int(ch_w[q])
            for half, hm, stX, spX, adX in ((0, hasA, stA, spA, addA),
                                            (1, hasB, stB, spB, addB)):
                if not hm[q]:
                    continue
                wf = 2 * w + half
                if stX[q]:
                    open_ps[wf] = cx.psp.tile(
                        [P, D], F32, space="PSUM",
                        tag=f"pseg{half}", name=f"pseg{half}", bufs=2)
                pt = open_ps[wf]
                nc.tensor.matmul(out=pt[:],
                                 lhsT=sel[half][:, k * P:(k + 1) * P],
                                 rhs=gv[:, k * D:(k + 1) * D],
                                 start=bool(stX[q]), stop=bool(spX[q]))
                if spX[q]:
                    t, sl = raw_put(wf)
                    if adX[q]:
                        nc.vector.tensor_add(t[:, sl], t[:, sl], pt[:])
                    else:
                        nc.scalar.activation(t[:, sl], pt[:], AF.Copy)
                    del open_ps[wf]
        if ag_after and bi in ag_after:
            for go in ag_after[bi]:
                go()
    assert not open_ps, f"{name}: unclosed psum groups {list(open_ps)}"


def emit_epilogue(cx, blocks, acc_sb, nprefix):
    """acc[:, w] += raw[:, w]/max(||raw_w||,1e-12) for windows 0..nprefix-1.
    blocks: list of (tile, nwin_in_tile)."""
    nc = cx.nc
    done = 0
    for (t, bw) in blocks:
        off = 0
        while off < bw and done < nprefix:
            ng = min(32, bw - off, nprefix - done)
            sl = slice(off * D, (off + ng) * D)
            sq = cx.ep.tile([P, 32 * D], F32, tag="ep_sq", name="ep_sq")
            nc.vector.tensor_mul(sq[:, :ng * D], t[:, sl], t[:, sl])
            ss = cx.ep.tile([P, 32], F32, tag="ep_ss", name="ep_ss")
            nc.vector.reduce_sum(ss[:, :ng], sq[:, :ng * D].rearrange("p (w d) -> p w d", w=ng),
                                 axis=mybir.AxisListType.X)
            snorm = cx.ep.tile([P, 32], F32, tag="ep_sn", name="ep_sn")
            nc.scalar.activation(snorm[:, :ng], ss[:, :ng], AF.Sqrt)
            nc.vector.tensor_scalar_max(snorm[:, :ng], snorm[:, :ng], 1e-12)
            rn = cx.ep.tile([P, 32], F32, tag="ep_rn", name="ep_rn")
            nc.vector.reciprocal(rn[:, :ng], snorm[:, :ng])
            contrib = cx.ep.tile([P, 32 * D], F32, tag="ep_ct", name="ep_ct")
            nc.vector.tensor_mul(
                contrib[:, :ng * D].rearrange("p (w d) -> p w d", w=ng),
                t[:, sl].rearrange("p (w d) -> p w d", w=ng),
                rn[:, :ng].to_broadcast([P, ng, D]),
            )
            nc.vector.tensor_add(acc_sb[:, done * D:(done + ng) * D],
                                 acc_sb[:, done * D:(done + ng) * D],
                                 contrib[:, :ng * D])
            done += ng
            off += ng


def indirect_gather_rows(cx, out_sb, table_ap, idx_sb, ncols):
    nc = cx.nc
    for k in range(ncols):
        nc.gpsimd.indirect_dma_start(
            out=out_sb[:, k * D:(k + 1) * D],
            out_offset=None,
            in_=table_ap,
            in_offset=bass.IndirectOffsetOnAxis(ap=idx_sb[:, k:k + 1], axis=0),
        )


def normalize_rows(cx, x_sb, ngroups, tag):
    nc = cx.nc
    sq = cx.lp.tile([P, ngroups * D], F32, tag=f"{tag}_sq")
    nc.vector.tensor_mul(sq[:], x_sb[:, :ngroups * D], x_sb[:, :ngroups * D])
    ss = cx.lp.tile([P, ngroups], F32, tag=f"{tag}_ss")
    nc.vector.reduce_sum(ss[:], sq[:].rearrange("p (w d) -> p w d", w=ngroups),
                         axis=mybir.AxisListType.X)
    sn = cx.lp.tile([P, ngroups], F32, tag=f"{tag}_sn")
    nc.scalar.activation(sn[:], ss[:], AF.Sqrt)
    nc.vector.tensor_scalar_max(sn[:], sn[:], 1e-12)
    rn = cx.lp.tile([P, ngroups], F32, tag=f"{tag}_rn")
    nc.vector.reciprocal(rn[:], sn[:])
    nc.vector.tensor_mul(
        x_sb[:, :ngroups * D].rearrange("p (w d) -> p w d", w=ngroups),
        x_sb[:, :ngroups * D].rearrange("p (w d) -> p w d", w=ngroups),
        rn[:].to_broadcast([P, ngroups, D]),
    )


def rowdot(cx, a_sb, b_sb, out_sb, ngroups, tag):
    nc = cx.nc
    t = cx.lp.tile([P, ngroups * D], F32, tag=f"{tag}_t")
    nc.vector.tensor_mul(t[:], a_sb[:, :ngroups * D], b_sb[:, :ngroups * D])
    nc.vector.reduce_sum(out_sb[:, :ngroups], t[:].rearrange("p (w d) -> p w d", w=ngroups),
                         axis=mybir.AxisListType.X)


def transpose_groups(cx, src_sb, ngroups, tag):
    nc = cx.nc
    out = cx.lp.tile([P, ngroups * P], F32, tag=f"{tag}_T")
    for g in range(ngroups):
        pt = cx.psp.tile([P, P], F32, space="PSUM", tag="tr_ps", bufs=1)
        nc.tensor.transpose(out=pt[:D, :P], in_=src_sb[:, g * D:(g + 1) * D],
                            identity=cx.ident[:])
        nc.vector.tensor_copy(out[:D, g * P:(g + 1) * P], pt[:D, :P])
    return out


def build(pp):
    dims = pp["dims"]
    R_acc1, R_f11 = dims["R_acc1"], dims["R_f11"]
    R_acc2, R_f12 = dims["R_acc2"], dims["R_f12"]
    RB = dims["RB"]
    blks_f11, blks_f12 = dims["blks_f11"], dims["blks_f12"]
    nsrc_ag = dims["nsrc_ag"]
    st = pp["streams"]

    nc = bacc.Bacc("TRN2", target_bir_lowering=False, debug=False,
                   num_devices=NCORES, num_swdge_queues=4)
    cx = Ctx()
    cx.nc = nc
    cx.qrr = 0
    cx.seli = 0

    # ---- dram inputs
    f0_acc_il_t = nc.dram_tensor("f0_acc_il", [R_acc1, D], F32, kind="ExternalInput")
    f0_acc_bl_t = nc.dram_tensor("f0_acc_bl", [R_acc2, D], F32, kind="ExternalInput")
    g_in = {}
    for nm in ("il1", "bl1", "il2", "bl2", "ag"):
        s = st[nm]
        tch = s["tch"]
        d = dict(
            lrA=nc.dram_tensor(f"{nm}_lrA", [128, tch], BF, kind="ExternalInput"),
            lrB=nc.dram_tensor(f"{nm}_lrB", [128, tch], BF, kind="ExternalInput"),
            val=nc.dram_tensor(f"{nm}_val", [128, tch],
                               BF if nm in ("il1", "bl1") else F32,
                               kind="ExternalInput"),
        )
        if nm in ("il1", "bl1"):
            d["pay"] = nc.dram_tensor(f"{nm}_pay", [128, tch * D], BF, kind="ExternalInput")
        else:
            d["idx"] = nc.dram_tensor(f"{nm}_idx", [128, tch * 8], I16, kind="ExternalInput")
        g_in[nm] = d
    debug = bool(int(os.environ.get("DSCBR_DEBUG", "0")))
    lidx = {k: nc.dram_tensor(f"loss_{k}", [128, v.shape[1]], I32, kind="ExternalInput")
            for k, v in pp["loss"][0].items()}
    aug_in = {k: nc.dram_tensor(k, [128, 16], I32, kind="ExternalInput")
              for k in ("aug_u_bl", "aug_b0_bl", "aug_b0_il")}
    out_t = nc.dram_tensor("out", [1, 2], F32, kind="ExternalOutput")

    with tile.TileContext(nc) as tc:
        cx.tc = tc
        es = []
        def pool(name, bufs, **kw):
            p = tc.tile_pool(name=name, bufs=bufs, **kw)
            es.append(p)
            return p.__enter__()
        cx.psp = pool("psum", 1, space="PSUM")
        cx.dramp = pool("dram", 1, space="DRAM")
        cx.cp = pool("const", 1)

        iota_i = cx.cp.tile([P, P], I32)
        nc.gpsimd.iota(iota_i[:], pattern=[[1, P]], base=0, channel_multiplier=0)
        cx.iota_bf = cx.cp.tile([P, P], BF)
        nc.vector.tensor_copy(cx.iota_bf[:], iota_i[:])
        cx.ident = cx.cp.tile([P, P], F32)
        make_identity(nc, cx.ident[:])
        ones_col = cx.cp.tile([P, 1], F32)
        nc.vector.memset(ones_col[:], 1.0)

        # ---------- scoped pools for the SpMM phases ----------
        es2 = []
        def pool2(name, bufs, **kw):
            p = tc.tile_pool(name=name, bufs=bufs, **kw)
            es2.append(p)
            return p.__enter__()
        cx.gsp = pool2("gstream", 6)
        cx.gp = pool2("gather", 8)
        cx.gvp = pool2("gval", 3)
        cx.idxp = pool2("gidx", 8)
        cx.selp = pool2("sel", 2)
        cx.ep = pool2("epil", 1)
        cx.mp = pool2("meta", 1)
        cx.accp = pool2("accs", 1)

        cx.meta = {}
        def load_meta(nm):
            s = st[nm]
            tch = s["tch"]
            vt = BF if nm in ("il1", "bl1") else F32
            lrA = cx.mp.tile([128, tch], BF, tag=f"{nm}_lrA", name=f"{nm}_lrA")
            lrB = cx.mp.tile([128, tch], BF, tag=f"{nm}_lrB", name=f"{nm}_lrB")
            vv = cx.mp.tile([128, tch], vt, tag=f"{nm}_vv", name=f"{nm}_vv")
            nc.sync.dma_start(out=lrA[:], in_=g_in[nm]["lrA"][:])
            nc.sync.dma_start(out=lrB[:], in_=g_in[nm]["lrB"][:])
            nc.sync.dma_start(out=vv[:], in_=g_in[nm]["val"][:])
            cx.meta[nm] = (lrA, lrB, vv)

        # acc buffers
        nacc1, nacc2 = R_acc1 // P, R_acc2 // P
        acc_il = cx.accp.tile([P, nacc1 * D], F32, tag="acc_il", name="acc_il")
        nc.sync.dma_start(out=acc_il[:].rearrange("p (w d) -> p w d", w=nacc1),
                          in_=f0_acc_il_t[:].rearrange("(w p) d -> p w d", p=P))
        acc_bl = cx.accp.tile([P, nacc2 * D], F32, tag="acc_bl", name="acc_bl")
        nc.sync.dma_start(out=acc_bl[:].rearrange("p (w d) -> p w d", w=nacc2),
                          in_=f0_acc_bl_t[:].rearrange("(w p) d -> p w d", p=P))

        # raw block tiles (32 windows each), shared by il1/bl1
        nblk = max(len(blks_f11), len(blks_f12))
        def raw_blocks(blks):
            tiles = []
            for i, bs in enumerate(blks):
                t = cx.accp.tile([P, 32 * D], F32, tag=f"rawblk{i}", name=f"rawblk{i}")
                tiles.append((t, bs // P))
            return tiles
        def raw_put_blocks(tiles):
            def put(wf):
                return tiles[wf // 32][0], slice((wf % 32) * D, (wf % 32 + 1) * D)
            return put

        # collective helper
        def ag_pair(nm, rows_in, rows_out):
            ain = cx.dramp.tile([rows_in, D], F32, tag=f"{nm}_agin", name=f"{nm}_agin")
            aout = cx.dramp.tile([rows_out, D], F32, addr_space="Shared",
                                 tag=f"{nm}_agout", name=f"{nm}_agout")
            return ain, aout

        def emit_l1(nm, blks, pay_t, last_batch_of_blk):
            tiles = raw_blocks(blks)
            outs = []
            ag_after = {}
            for i, bs in enumerate(blks):
                ain, aout = ag_pair(f"{nm}b{i}", bs, bs * NCORES)
                outs.append(aout)
                def mk(i=i, bs=bs, ain=ain, aout=aout):
                    def go():
                        t, nw = tiles[i]
                        nc.sync.dma_start(
                            out=ain[:].rearrange("(w p) d -> p w d", p=P),
                            in_=t[:, :nw * D].rearrange("p (w d) -> p w d", w=nw))
                        nc.gpsimd.collective_compute(
                            "AllGather", ALU.bypass,
                            replica_groups=[list(range(NCORES))],
                            ins=[ain[:].opt()], outs=[aout[:].opt()])
                    return go
                ag_after.setdefault(last_batch_of_blk[i], []).append(mk())
            emit_graph(cx, nm, st[nm], None, pay_t, None,
                       raw_put_blocks(tiles), ag_after=ag_after)
            return tiles, outs

        def last_batches(s, blks):
            """batch index after which each block's drains are complete."""
            nbat = len(s["batches"])
            out = []
            for i in range(len(blks)):
                wlo, whi = (sum(b // P for b in blks[:i]),
                            sum(b // P for b in blks[:i + 1]))
                lb = 0
                for bi, (sg, q0, nch) in enumerate(s["batches"]):
                    for q in range(q0, q0 + nch):
                        w = int(s["ch_w"][q])
                        if wlo <= 2 * w < whi or wlo <= 2 * w + 1 < whi:
                            if s["wp_last_q"][w] == q:
                                lb = bi
                out.append(lb)
            return out

        # wp_last_q helper array on streams
        for nm in ("il1", "bl1", "il2", "bl2", "ag"):
            s = st[nm]
            wpl = np.full(s["nwp"], -1, np.int64)
            for w in range(s["nwp"]):
                wpl[w] = max(s["lastq"][2 * w], s["lastq"][2 * w + 1])
            s["wp_last_q"] = wpl

        # ---------------- il1 ----------------
        for nm in ("il1", "bl1", "il2", "bl2", "ag"):
            load_meta(nm)
        il1_tiles, f1_il_blks = emit_l1("il1", blks_f11, g_in["il1"]["pay"],
                                        last_batches(st["il1"], blks_f11))
        if debug:
            raw_dump = nc.dram_tensor("dbg_raw_il1b0", [blks_f11[0], D], F32,
                                      kind="ExternalOutput")
            t0, nw0 = il1_tiles[0]
            nc.sync.dma_start(out=raw_dump[:].rearrange("(w p) d -> p w d", p=P),
                              in_=t0[:, :nw0 * D].rearrange("p (w d) -> p w d", w=nw0))
        emit_epilogue(cx, il1_tiles, acc_il, nacc1)

        # ---------------- bl1 ----------------
        bl1_tiles, f1_bl_blks = emit_l1("bl1", blks_f12, g_in["bl1"]["pay"],
                                        last_batches(st["bl1"], blks_f12))
        emit_epilogue(cx, bl1_tiles, acc_bl, nacc2)

        # ---------------- il2 ----------------
        raw2 = cx.accp.tile([P, nacc1 * D], F32, tag="raw2", name="raw2")
        def raw2_put(wf):
            return raw2, slice(wf * D, (wf + 1) * D)
        emit_graph(cx, "il2", st["il2"], [t[:] for t in f1_il_blks], None,
                   g_in["il2"]["idx"], raw2_put)
        emit_epilogue(cx, [(raw2, nacc1)], acc_il, nacc1)
        acc_il_in, acc_il_full = ag_pair("accil", R_acc1, R_acc1 * NCORES)
        nc.sync.dma_start(out=acc_il_in[:].rearrange("(w p) d -> p w d", p=P),
                          in_=acc_il[:].rearrange("p (w d) -> p w d", w=nacc1))
        nc.gpsimd.collective_compute(
            "AllGather", ALU.bypass, replica_groups=[list(range(NCORES))],
            ins=[acc_il_in[:].opt()], outs=[acc_il_full[:].opt()])

        # ---------------- bl2 ----------------
        raw2b = cx.accp.tile([P, nacc2 * D], F32, tag="raw2b", name="raw2b")
        def raw2b_put(wf):
            return raw2b, slice(wf * D, (wf + 1) * D)
        emit_graph(cx, "bl2", st["bl2"], [t[:] for t in f1_bl_blks], None,
                   g_in["bl2"]["idx"], raw2b_put)
        emit_epilogue(cx, [(raw2b, nacc2)], acc_bl, nacc2)
        acc_bl_in, acc_bl_full = ag_pair("accbl", R_acc2, R_acc2 * NCORES)
        nc.sync.dma_start(out=acc_bl_in[:].rearrange("(w p) d -> p w d", p=P),
                          in_=acc_bl[:].rearrange("p (w d) -> p w d", w=nacc2))
        nc.gpsimd.collective_compute(
            "AllGather", ALU.bypass, replica_groups=[list(range(NCORES))],
            ins=[acc_bl_in[:].opt()], outs=[acc_bl_full[:].opt()])

        # ---------------- agg ----------------
        nwB = RB // P
        rawag = cx.accp.tile([P, nwB * D], F32, tag="rawag", name="rawag")
        def rawag_put(wf):
            return rawag, slice(wf * D, (wf + 1) * D)
        acc_il_ap = acc_il_full[:]
        ag_tables = [acc_il_ap[s * SRC_WIN: min((s + 1) * SRC_WIN, R_acc1 * NCORES), :]
                     for s in range(nsrc_ag)]
        emit_graph(cx, "ag", st["ag"], ag_tables, None, g_in["ag"]["idx"], rawag_put)
        ilb_in, ilb_full = ag_pair("ilb", RB, RB * NCORES)
        nc.sync.dma_start(out=ilb_in[:].rearrange("(w p) d -> p w d", p=P),
                          in_=rawag[:, :nwB * D].rearrange("p (w d) -> p w d", w=nwB))
        nc.gpsimd.collective_compute(
            "AllGather", ALU.bypass, replica_groups=[list(range(NCORES))],
            ins=[ilb_in[:].opt()], outs=[ilb_full[:].opt()])

        if debug:
            for nm, t, rows in (("dbg_acc_il", acc_il_full, R_acc1 * NCORES),
                                ("dbg_acc_bl", acc_bl_full, R_acc2 * NCORES),
                                ("dbg_ilb", ilb_full, RB * NCORES),
                                ("dbg_f1il0", f1_il_blks[0], blks_f11[0] * NCORES)):
                o = nc.dram_tensor(nm, [rows, D], F32, kind="ExternalOutput")
                nc.sync.dma_start(out=o[:], in_=t[:])

        for p in reversed(es2):
            p.__exit__(None, None, None)
        cx.lp = pool("loss", 1)

        # ---------------- loss ----------------
        bsh = BATCH // NCORES          # 256
        ng = bsh // P                  # 2
        lidx_sb = {}
        for k, t in lidx.items():
            s = cx.lp.tile([128, t.shape[1]], I32, tag=f"li_{k}")
            nc.sync.dma_start(out=s[:], in_=t[:])
            lidx_sb[k] = s
        for k, t in aug_in.items():
            s = cx.lp.tile([128, 16], I32, tag=f"li_{k}")
            nc.sync.dma_start(out=s[:], in_=t[:])
            lidx_sb[k] = s

        def gather(tag, table, idxk, ncols):
            sb = cx.lp.tile([P, ncols * D], F32, tag=tag)
            indirect_gather_rows(cx, sb, table, lidx_sb[idxk], ncols)
            return sb
        pos_u_il = gather("pos_u_il", acc_il_full[:], "u_il", ng)
        pos_u_bl = gather("pos_u_bl", acc_bl_full[:], "u_bl", ng)
        b_il0 = gather("b_il0", ilb_full[:], "b_il0", ng)
        b_il1 = gather("b_il1", ilb_full[:], "b_il1", ng)
        b_bl0 = gather("b_bl0", acc_bl_full[:], "b_bl0", ng)
        b_bl1 = gather("b_bl1", acc_bl_full[:], "b_bl1", ng)
        aug_u = gather("aug_u", acc_bl_full[:], "aug_u_bl", 16)
        aug_b = gather("aug_b", acc_bl_full[:], "aug_b0_bl", 16)
        # -- bpr
        pr0 = cx.lp.tile([P, ng], F32, tag="pr0")
        pr1 = cx.lp.tile([P, ng], F32, tag="pr1")
        tmp = cx.lp.tile([P, ng], F32, tag="prt")
        rowdot(cx, pos_u_il, b_il0, pr0, ng, "d0")
        rowdot(cx, pos_u_bl, b_bl0, tmp, ng, "d1")
        nc.vector.tensor_add(pr0[:], pr0[:], tmp[:])
        rowdot(cx, pos_u_il, b_il1, pr1, ng, "d2")
        rowdot(cx, pos_u_bl, b_bl1, tmp, ng, "d3")
        nc.vector.tensor_add(pr1[:], pr1[:], tmp[:])
        x = cx.lp.tile([P, ng], F32, tag="bprx")
        nc.vector.tensor_tensor(out=x[:], in0=pr1[:], in1=pr0[:], op=ALU.subtract)
        negx = cx.lp.tile([P, ng], F32, tag="bprnx")
        nc.vector.tensor_scalar_mul(negx[:], x[:], -1.0)
        nax = cx.lp.tile([P, ng], F32, tag="bprax")
        nc.vector.tensor_tensor(out=nax[:], in0=x[:], in1=negx[:], op=ALU.min)
        e = cx.lp.tile([P, ng], F32, tag="bpre")
        nc.scalar.activation(e[:], nax[:], AF.Exp)
        nc.vector.tensor_scalar_add(e[:], e[:], 1.0)
        l1p = cx.lp.tile([P, ng], F32, tag="bprl")
        nc.scalar.activation(l1p[:], e[:], AF.Ln)
        sp = cx.lp.tile([P, ng], F32, tag="bprsp")
        nc.vector.tensor_scalar_max(sp[:], x[:], 0.0)
        nc.vector.tensor_add(sp[:], sp[:], l1p[:])

        normalize_rows(cx, aug_u, 16, "nau")
        normalize_rows(cx, aug_b, 16, "nab")
        normalize_rows(cx, pos_u_il, ng, "npu")
        my_pos_b = cx.lp.tile([P, ng * D], F32, tag="my_pb")
        nc.vector.tensor_copy(my_pos_b[:], b_il0[:, :ng * D])
        normalize_rows(cx, my_pos_b, ng, "npb")

        part = cx.lp.tile([P, 4], F32, tag="parts")
        nc.vector.memset(part[:], 0.0)
        nc.vector.reduce_sum(part[:, 0:1], sp[:].rearrange("p (w d) -> p w d", w=1),
                             axis=mybir.AxisListType.X)

        def closs_partial(pos_my, aug_full, aug_my_cols, out_col):
            posT = transpose_groups(cx, pos_my, ng, "pT")
            augT = transpose_groups(cx, aug_full, 16, "aT")
            ps = cx.lp.tile([P, ng], F32, tag="ps")
            rowdot(cx, pos_my, aug_my_cols, ps, ng, f"psd{out_col}")
            lse = cx.lp.tile([P, ng], F32, tag="lse")
            for g in range(ng):
                ttl_ps = cx.psp.tile([P, 512], F32, space="PSUM", tag="ttl", bufs=1)
                ttl = cx.lp.tile([P, BATCH], F32, tag="ttl")
                for nb_ in range(BATCH // 512):
                    nc.tensor.matmul(
                        out=ttl_ps[:, :512],
                        lhsT=posT[:D, g * P:(g + 1) * P],
                        rhs=augT[:D, nb_ * 512:(nb_ + 1) * 512],
                        start=True, stop=True)
                    nc.vector.tensor_copy(ttl[:, nb_ * 512:(nb_ + 1) * 512], ttl_ps[:, :512])
                mx = cx.lp.tile([P, 1], F32, tag="mx")
                nc.vector.reduce_max(mx[:], ttl[:].rearrange("p (w d) -> p w d", w=1),
                                     axis=mybir.AxisListType.X)
                nmx = cx.lp.tile([P, 1], F32, tag="nmx")
                nc.vector.tensor_scalar_mul(nmx[:], mx[:], -4.0)
                ex = cx.lp.tile([P, BATCH], F32, tag="ex")
                se = cx.lp.tile([P, 1], F32, tag="se")
                nc.scalar.activation(ex[:], ttl[:], AF.Exp, bias=nmx[:, :1], scale=4.0,
                                     accum_out=se[:, :1])
                ln = cx.lp.tile([P, 1], F32, tag="ln")
                nc.scalar.activation(ln[:], se[:], AF.Ln)
                m4 = cx.lp.tile([P, 1], F32, tag="m4")
                nc.vector.tensor_scalar_mul(m4[:], mx[:], 4.0)
                nc.vector.tensor_add(lse[:, g:g + 1], ln[:], m4[:])
            t4 = cx.lp.tile([P, ng], F32, tag="t4")
            nc.vector.tensor_scalar_mul(t4[:], ps[:], 4.0)
            nc.vector.tensor_tensor(out=t4[:], in0=t4[:], in1=lse[:], op=ALU.subtract)
            nc.vector.reduce_sum(part[:, out_col:out_col + 1],
                                 t4[:].rearrange("p (w d) -> p w d", w=1),
                                 axis=mybir.AxisListType.X)

        aug_u_my = gather("aug_u_my", acc_bl_full[:], "u_bl", ng)
        normalize_rows(cx, aug_u_my, ng, "naum")
        aug_b_my = gather("aug_b_my", acc_bl_full[:], "b_bl0", ng)
        normalize_rows(cx, aug_b_my, ng, "nabm")
        closs_partial(pos_u_il, aug_u, aug_u_my, 1)
        closs_partial(my_pos_b, aug_b, aug_b_my, 2)

        pp_ps = cx.psp.tile([P, 4], F32, space="PSUM", tag="ppps", bufs=1)
        nc.tensor.matmul(out=pp_ps[:1, :4], lhsT=ones_col[:], rhs=part[:],
                         start=True, stop=True)
        psum_sb = cx.lp.tile([1, 4], F32, tag="psums")
        nc.vector.tensor_copy(psum_sb[:], pp_ps[:1, :4])
        ar_in = cx.dramp.tile([1, 4], F32, tag="ar_in")
        ar_out = cx.dramp.tile([1, 4], F32, addr_space="Shared", tag="ar_out")
        nc.sync.dma_start(out=ar_in[:], in_=psum_sb[:])
        nc.gpsimd.collective_compute(
            "AllReduce", ALU.add, replica_groups=[list(range(NCORES))],
            ins=[ar_in[:].opt()], outs=[ar_out[:].opt()])
        fin = cx.lp.tile([1, 4], F32, tag="fin")
        nc.sync.dma_start(out=fin[:], in_=ar_out[:])
        res = cx.lp.tile([1, 2], F32, tag="res")
        nc.vector.tensor_scalar_mul(res[:, 0:1], fin[:, 0:1], 1.0 / BATCH)
        t = cx.lp.tile([1, 1], F32, tag="rt")
        nc.vector.tensor_add(t[:], fin[:, 1:2], fin[:, 2:3])
        nc.vector.tensor_scalar_mul(res[:, 1:2], t[:], -0.5 / BATCH)
        nc.sync.dma_start(out=out_t[:], in_=res[:])

        for p in reversed(es):
            p.__exit__(None, None, None)
    nc.compile()
    return nc


# ---------------------------------------------------------------- entry point

def _install_ntff_hook():
    if "antenv.axon_hooks" in sys.modules:
        return
    try:
        mod = types.ModuleType("antenv.axon_hooks")
        _hook = [None]
        mod.set_axon_ntff_profile_hook = lambda h: _hook.__setitem__(0, h)
        mod.get_axon_ntff_profile_hook = lambda: _hook[0]
        sys.modules["antenv.axon_hooks"] = mod
        import antenv
        antenv.axon_hooks = mod
        from trn_agent_boot.trn_boot import _ntff_profile_via_ctypes
        hook = _ntff_profile_via_ctypes("/opt/axon/libaxon_pjrt.so")
        if hook is not None:
            mod.set_axon_ntff_profile_hook(hook)
    except Exception:
        pass


def make_in_maps(pp):
    import ml_dtypes
    maps = []
    st = pp["streams"]
    for c in range(NCORES):
        m = {
            "f0_acc_il": pp["f0_acc"]["il"][c],
            "f0_acc_bl": pp["f0_acc"]["bl"][c],
            "aug_u_bl": pp["aug_u_bl"], "aug_b0_bl": pp["aug_b0_bl"],
            "aug_b0_il": pp["aug_b0_il"],
        }
        for nm in ("il1", "bl1", "il2", "bl2", "ag"):
            s = st[nm]
            m[f"{nm}_lrA"] = np.ascontiguousarray(
                s["lrA"][c].reshape(-1, P).T).astype(ml_dtypes.bfloat16)
            m[f"{nm}_lrB"] = np.ascontiguousarray(
                s["lrB"][c].reshape(-1, P).T).astype(ml_dtypes.bfloat16)
            vals = np.ascontiguousarray(s["val"][c].reshape(-1, P).T)
            if nm in ("il1", "bl1"):
                m[f"{nm}_val"] = vals.astype(ml_dtypes.bfloat16)
                m[f"{nm}_pay"] = pp["pay"][nm][c]
            else:
                m[f"{nm}_val"] = vals
                m[f"{nm}_idx"] = wrap_idx16(s["srcv"][c].astype(np.int16))
        for k, v in pp["loss"][c].items():
            m[f"loss_{k}"] = v
        maps.append(m)
    return maps


_CACHE = {}


def kernel(**inputs) -> np.ndarray:
    _install_ntff_hook()
    pp = preprocess(inputs)
    key = tuple(sorted((k, v) for k, v in pp["dims"].items()
                       if isinstance(v, int)))
    if key not in _CACHE:
        _CACHE[key] = build(pp)
    nc = _CACHE[key]
    in_maps = make_in_maps(pp)
    trace = bool(int(os.environ.get("DSCBR_TRACE", "0")))
    res = run_bass_kernel_spmd(nc, in_maps, core_ids=list(range(NCORES)), trace=trace)
    if trace and res.exec_time_ns:
        print(f"HW exec time: {res.exec_time_ns} ns")
    kernel._last_results = res.results
    out = res.results[0]["out"].reshape(2).astype(np.float32)
    return out


# revision 50
# speedup vs baseline: 1.0806x; 1.0370x over previous
"""Trainium2 Bass kernel for nn_DSCBR (gnn_message_passing).

Strategy (8 NeuronCores, SPMD, dest-sharded):
- Host prunes both propagation graphs by backward slicing from the loss batch
  (only rows that feed the final losses are computed), then compacts each
  layer's destination space; compact rows are round-robin sharded.
- Layer-1 SpMM sources come from the INPUT feature tables, so the host
  pre-gathers them into per-core edge-ordered payload streams (pure indexing;
  all FP math stays on device).  The device streams payloads contiguously
  (HWDGE), multiplies by edge values, and segment-sums via selection-matrix
  matmuls accumulated in PSUM bank tiles.
- Layer-2/agg SpMM sources are runtime tables; gathered per edge with
  dma_gather spread over 4 SWDGE queues.
- f1 tables are all-gathered in per-block sub-collectives so layer-2 can
  start on block 0 while later blocks are still in flight.
- Losses (BPR + two contrastive views) computed batch-sharded + AllReduce.
"""
import os
import sys
import types

sys.path.insert(0, "/opt/trn_rl_repo")

import numpy as np

import concourse.bass as bass
import concourse.bacc as bacc
import concourse.mybir as mybir
import concourse.tile as tile
from concourse.bass_utils import run_bass_kernel_spmd
from concourse.masks import make_identity

P = 128
NCORES = 8
SRC_WIN = 32768
BLK_SLOTS = SRC_WIN // NCORES   # 4096 per-core slots per AG block
GI_MAX = 2048
D = 64
NU, NI, NB = 100000, 50000, 20000
BATCH = 2048
F32 = mybir.dt.float32
I32 = mybir.dt.int32
I16 = mybir.dt.int16
BF = mybir.dt.bfloat16
AF = mybir.ActivationFunctionType
ALU = mybir.AluOpType


# ---------------------------------------------------------------- host prep

def _pad_ids(real, n_space, mult):
    """real: sorted unique ids. Append complement ids to a multiple of mult."""
    need = (-len(real)) % mult
    if need == 0:
        return np.asarray(real, np.int64)
    m = np.ones(n_space, bool)
    m[real] = False
    pad = np.flatnonzero(m)[:need]
    assert len(pad) == need, "no room to pad id set"
    return np.concatenate([np.asarray(real, np.int64), pad])


def _pad_concat(base, extra, n_space, mult):
    arr = np.concatenate([np.asarray(base, np.int64), np.asarray(extra, np.int64)])
    need = (-len(arr)) % mult
    if need == 0:
        return arr
    m = np.ones(n_space, bool)
    m[arr] = False
    pad = np.flatnonzero(m)[:need]
    assert len(pad) == need
    return np.concatenate([arr, pad])


def _posmap(ids, n_space):
    g = np.full(n_space, -1, np.int64)
    g[ids] = np.arange(len(ids))
    return g


def _blk_sizes(R):
    """Per-core block slot counts (multiples of 128), blocks of <=BLK_SLOTS."""
    out = []
    left = R
    while left > 0:
        out.append(min(BLK_SLOTS, left))
        left -= out[-1]
    return out


def _runs(mask):
    """Maximal [a,b) runs of True in a 1-d bool array."""
    out = []
    a = None
    for i, v in enumerate(mask):
        if v and a is None:
            a = i
        elif not v and a is not None:
            out.append((a, i))
            a = None
    if a is not None:
        out.append((a, len(mask)))
    return out


def build_stream(rows_pos, swin, srcv, vals, R, nsrc, ncores=NCORES):
    """Build per-core dest-sharded edge streams with (swin, win-pair, lrow)
    grouping, 128-edge chunks padded to the max count over cores.

    Returns a dict with per-core streams and the shared chunk program."""
    nwp = R // (2 * P)
    rows_pos = np.asarray(rows_pos, np.int64)
    swin = np.asarray(swin, np.int64)
    core = rows_pos % ncores
    slot = rows_pos // ncores
    wp = slot // (2 * P)
    lrow = slot % (2 * P)
    order = np.lexsort((lrow, wp, swin, core))
    c_s, s_s, w_s, l_s = core[order], swin[order], wp[order], lrow[order]
    sv_s, v_s = np.asarray(srcv)[order], np.asarray(vals)[order]

    counts = np.zeros((ncores, nsrc, nwp), np.int64)
    np.add.at(counts, (c_s, s_s, w_s), 1)
    countsA = np.zeros((ncores, nsrc, nwp), np.int64)
    mA = l_s < P
    np.add.at(countsA, (c_s[mA], s_s[mA], w_s[mA]), 1)
    maxc = counts.max(axis=0)
    nchunks = (maxc + P - 1) // P                     # [nsrc, nwp]
    tch = int(nchunks.sum())
    grp_choff = np.concatenate([[0], np.cumsum(nchunks.ravel())])[:-1].reshape(nsrc, nwp)

    key = (c_s * nsrc + s_s) * nwp + w_s
    starts = np.searchsorted(key, np.arange(ncores * nsrc * nwp))
    ends = np.searchsorted(key, np.arange(ncores * nsrc * nwp) + 1)

    srcv_st = np.zeros((ncores, tch * P), np.int64)
    lrA = np.full((ncores, tch * P), 300.0, np.float32)
    lrB = np.full((ncores, tch * P), 300.0, np.float32)
    val_st = np.zeros((ncores, tch * P), np.float32)
    ch_s = np.zeros(tch, np.int64)
    ch_w = np.zeros(tch, np.int64)
    hasA = np.zeros(tch, bool)
    hasB = np.zeros(tch, bool)
    for s in range(nsrc):
        for w in range(nwp):
            ncw = int(nchunks[s, w])
            if ncw == 0:
                continue
            off = int(grp_choff[s, w])
            ch_s[off:off + ncw] = s
            ch_w[off:off + ncw] = w
            for c in range(ncores):
                k = (c * nsrc + s) * nwp + w
                a, b = int(starts[k]), int(ends[k])
                n = b - a
                if n == 0:
                    continue
                pos = off * P
                srcv_st[c, pos:pos + n] = sv_s[a:b]
                lr = l_s[a:b]
                lrA[c, pos:pos + n] = np.where(lr < P, lr, 300.0)
                lrB[c, pos:pos + n] = np.where(lr >= P, lr - P, 300.0)
                val_st[c, pos:pos + n] = v_s[a:b]
                nAc = int(countsA[c, s, w])
                kA = (nAc + P - 1) // P
                hasA[off:off + kA] = True
                if n > nAc:
                    hasB[off + nAc // P: off + (n + P - 1) // P] = True

    nwin = R // P
    firstq = np.full(2 * nwp, -1, np.int64)
    lastq = np.full(2 * nwp, -1, np.int64)
    for q in range(tch):
        for half, has in ((0, hasA[q]), (1, hasB[q])):
            if has:
                wf = 2 * ch_w[q] + half
                if firstq[wf] < 0:
                    firstq[wf] = q
                lastq[wf] = q
    memset_wf = [wf for wf in range(nwin) if firstq[wf] < 0]

    # per-(segment, wp, half) start/stop flags and drain ops.
    # PSUM accumulation groups must not interleave within a bank, so each
    # (s, w, half) is its own group in its own tile; drains copy on the
    # half's first touched segment and add on later ones.
    stA = np.zeros(tch, bool)
    spA = np.zeros(tch, bool)
    stB = np.zeros(tch, bool)
    spB = np.zeros(tch, bool)
    addA = np.zeros(tch, bool)   # at spA chunk: accumulate into raw, not copy
    addB = np.zeros(tch, bool)
    seen_wf = set()
    for s in range(nsrc):
        for w in range(nwp):
            ncw = int(nchunks[s, w])
            if ncw == 0:
                continue
            off = int(grp_choff[s, w])
            qs = np.arange(off, off + ncw)
            for half, hm, stX, spX, adX in ((0, hasA, stA, spA, addA),
                                            (1, hasB, stB, spB, addB)):
                qa = qs[hm[qs]]
                if len(qa) == 0:
                    continue
                stX[qa[0]] = True
                spX[qa[-1]] = True
                wf = 2 * w + half
                if wf in seen_wf:
                    adX[qa[-1]] = True
                seen_wf.add(wf)

    batches = []
    for s in range(nsrc):
        lo = int(grp_choff[s, 0])
        hi = int(grp_choff[s, nwp - 1] + nchunks[s, nwp - 1]) if nwp else lo
        q0 = lo
        while q0 < hi:
            nch = min(GI_MAX // P, hi - q0)
            batches.append((s, q0, nch))
            q0 += nch

    return dict(srcv=srcv_st, lrA=lrA, lrB=lrB, val=val_st,
                ch_s=ch_s, ch_w=ch_w, hasA=hasA, hasB=hasB,
                firstq=firstq, lastq=lastq, memset_wf=memset_wf,
                stA=stA, spA=spA, stB=stB, spB=spB, addA=addA, addB=addB,
                batches=batches, tch=tch, nwp=nwp, R=R, nsrc=nsrc)


def wrap_idx16(flat):
    # index i -> partition i%16, col i//16; replicated x8 down partitions
    return np.ascontiguousarray(np.tile(flat.reshape(-1, 16).T.astype(np.int16), (8, 1)))


def idx_cols_i32(flat):
    # [n] -> [128, n/128] int32; col k = rows [128k, 128k+128)
    n = flat.shape[0]
    assert n % P == 0
    return np.ascontiguousarray(flat.reshape(-1, P).T.astype(np.int32))


def _table_pos(g, R):
    """compact position -> row in the core-slab-major all-gathered table."""
    return (g % NCORES) * R + g // NCORES


def _blk_pos(g, sizes):
    """compact position -> (block, idx within block) for block-major tables."""
    c, s = g % NCORES, g // NCORES
    k = s // BLK_SLOTS
    sz = np.asarray(sizes, np.int64)[k]
    return k, c * sz + (s - k * BLK_SLOTS)


def preprocess(inputs, ncores=NCORES):
    u = np.asarray(inputs["users_feature"], np.float32)
    it = np.asarray(inputs["items_feature"], np.float32)
    b = np.asarray(inputs["bundles_feature"], np.float32)
    f0_il = np.concatenate([u, it], 0)
    f0_bl = np.concatenate([u, b], 0)
    N1, N2 = NU + NI, NU + NB

    il_row = np.asarray(inputs["il_row"], np.int64)
    il_col = np.asarray(inputs["il_col"], np.int64)
    il_val = np.asarray(inputs["il_val"], np.float32)
    bl_row = np.asarray(inputs["bl_row"], np.int64)
    bl_col = np.asarray(inputs["bl_col"], np.int64)
    bl_val = np.asarray(inputs["bl_val"], np.float32)
    agg_row = np.asarray(inputs["agg_row"], np.int64)
    agg_col = np.asarray(inputs["agg_col"], np.int64)
    agg_val = np.asarray(inputs["agg_val"], np.float32)
    users = np.asarray(inputs["users"], np.int64)
    bundles = np.asarray(inputs["bundles"], np.int64)

    # ---- active sets (backward slice from the loss batch)
    mB = np.zeros(NB, bool)
    mB[bundles.ravel()] = True
    BstarP = _pad_ids(np.flatnonzero(mB), NB, 2048)
    gB = _posmap(BstarP, NB)
    keep_ag = mB[agg_row]
    items = np.unique(agg_col[keep_ag])
    uuniq = np.unique(users)

    # il graph
    S_acc_il = np.union1d(uuniq, NU + items)
    S_acc_ilP = _pad_ids(S_acc_il, N1, 2048)
    g_acc_il = _posmap(S_acc_ilP, N1)
    acc_mask_il = np.zeros(N1, bool)
    acc_mask_il[S_acc_il] = True
    keep2_il = acc_mask_il[il_row]
    S2_il = np.unique(il_col[keep2_il])
    inP = np.zeros(N1, bool)
    inP[S_acc_ilP] = True
    F1_ilP = _pad_concat(S_acc_ilP, S2_il[~inP[S2_il]], N1, 2048)
    g_f1_il = _posmap(F1_ilP, N1)
    f1_mask_il = acc_mask_il.copy()
    f1_mask_il[S2_il] = True
    keep1_il = f1_mask_il[il_row]

    # bl graph
    S_acc_bl = np.union1d(uuniq, NU + BstarP)
    S_acc_blP = _pad_ids(S_acc_bl, N2, 2048)
    g_acc_bl = _posmap(S_acc_blP, N2)
    acc_mask_bl = np.zeros(N2, bool)
    acc_mask_bl[S_acc_bl] = True
    keep2_bl = acc_mask_bl[bl_row]
    S2_bl = np.unique(bl_col[keep2_bl])
    inP2 = np.zeros(N2, bool)
    inP2[S_acc_blP] = True
    F1_blP = _pad_concat(S_acc_blP, S2_bl[~inP2[S2_bl]], N2, 2048)
    g_f1_bl = _posmap(F1_blP, N2)
    f1_mask_bl = acc_mask_bl.copy()
    f1_mask_bl[S2_bl] = True
    keep1_bl = f1_mask_bl[bl_row]

    R_acc1 = len(S_acc_ilP) // ncores
    R_f11 = len(F1_ilP) // ncores
    R_acc2 = len(S_acc_blP) // ncores
    R_f12 = len(F1_blP) // ncores
    RB = len(BstarP) // ncores
    blks_f11 = _blk_sizes(R_f11)
    blks_f12 = _blk_sizes(R_f12)

    # ---- edge streams
    il1 = build_stream(g_f1_il[il_row[keep1_il]], np.zeros(int(keep1_il.sum()), np.int64),
                       il_col[keep1_il], il_val[keep1_il], R_f11, 1)
    bl1 = build_stream(g_f1_bl[bl_row[keep1_bl]], np.zeros(int(keep1_bl.sum()), np.int64),
                       bl_col[keep1_bl], bl_val[keep1_bl], R_f12, 1)

    p2 = g_f1_il[il_col[keep2_il]]
    k2, i2 = _blk_pos(p2, blks_f11)
    il2 = build_stream(g_acc_il[il_row[keep2_il]], k2, i2,
                       il_val[keep2_il], R_acc1, len(blks_f11))
    p2b = g_f1_bl[bl_col[keep2_bl]]
    k2b, i2b = _blk_pos(p2b, blks_f12)
    bl2 = build_stream(g_acc_bl[bl_row[keep2_bl]], k2b, i2b,
                       bl_val[keep2_bl], R_acc2, len(blks_f12))

    pag = _table_pos(g_acc_il[NU + agg_col[keep_ag]], R_acc1)
    nsrc_ag = (ncores * R_acc1 + SRC_WIN - 1) // SRC_WIN
    ag = build_stream(gB[agg_row[keep_ag]], pag // SRC_WIN, pag % SRC_WIN,
                      agg_val[keep_ag], RB, nsrc_ag)

    # ---- L1 payloads (host pre-gather of INPUT features; edge-ordered)
    import ml_dtypes
    def payload(st, f0):
        out = []
        for c in range(ncores):
            arr = f0[st["srcv"][c]]                     # [tch*128, D]
            arr = arr.reshape(st["tch"], P, D).transpose(1, 0, 2).reshape(P, st["tch"] * D)
            out.append(np.ascontiguousarray(arr.astype(ml_dtypes.bfloat16)))
        return out
    il1_pay = payload(il1, f0_il)
    bl1_pay = payload(bl1, f0_bl)

    # ---- initial acc rows (f0 at compact acc rows, per core)
    f0_acc_il = [np.ascontiguousarray(f0_il[S_acc_ilP[c::ncores]]) for c in range(ncores)]
    f0_acc_bl = [np.ascontiguousarray(f0_bl[S_acc_blP[c::ncores]]) for c in range(ncores)]

    # ---- loss indices
    loss = {}
    bsh = BATCH // ncores
    for c in range(ncores):
        sl = slice(c * bsh, (c + 1) * bsh)
        loss[c] = dict(
            u_il=idx_cols_i32(_table_pos(g_acc_il[users[sl]], R_acc1)),
            u_bl=idx_cols_i32(_table_pos(g_acc_bl[users[sl]], R_acc2)),
            b_il0=idx_cols_i32(_table_pos(gB[bundles[sl, 0]], RB)),
            b_il1=idx_cols_i32(_table_pos(gB[bundles[sl, 1]], RB)),
            b_bl0=idx_cols_i32(_table_pos(g_acc_bl[bundles[sl, 0] + NU], R_acc2)),
            b_bl1=idx_cols_i32(_table_pos(g_acc_bl[bundles[sl, 1] + NU], R_acc2)),
        )
    aug_u_bl = idx_cols_i32(_table_pos(g_acc_bl[users], R_acc2))
    aug_b0_bl = idx_cols_i32(_table_pos(g_acc_bl[bundles[:, 0] + NU], R_acc2))
    aug_b0_il = idx_cols_i32(_table_pos(gB[bundles[:, 0]], RB))

    return dict(streams=dict(il1=il1, bl1=bl1, il2=il2, bl2=bl2, ag=ag),
                pay=dict(il1=il1_pay, bl1=bl1_pay),
                f0_acc=dict(il=f0_acc_il, bl=f0_acc_bl),
                loss=loss, aug_u_bl=aug_u_bl, aug_b0_bl=aug_b0_bl,
                aug_b0_il=aug_b0_il,
                dims=dict(R_acc1=R_acc1, R_f11=R_f11, R_acc2=R_acc2,
                          R_f12=R_f12, RB=RB, blks_f11=blks_f11,
                          blks_f12=blks_f12, nsrc_ag=nsrc_ag))


# ---------------------------------------------------------------- bass build

class Ctx:
    pass


NPB = 5  # psum bank tiles (8 windows each)


def emit_graph(cx, name, st, src_tables, pay_dram, idx_dram, raw_put, ag_after=None):
    """Emit one SpMM layer.

    src_tables: list of table APs per source window (gather mode), or None.
    pay_dram: payload dram tensor (stream mode), or None.
    raw_put(wf) -> (tile, col_slice) drain destination.
    ag_after: optional dict {batch_index: callable} to emit sub-AG right
    after that batch (pipelined collectives).
    """
    nc = cx.nc
    lrA_sb, lrB_sb, val_sb = cx.meta[name]
    for wf in st["memset_wf"]:
        t, sl = raw_put(wf)
        nc.vector.memset(t[:, sl], 0.0)

    hasA, hasB = st["hasA"], st["hasB"]
    ch_w = st["ch_w"]
    stA, spA, stB, spB = st["stA"], st["spA"], st["stB"], st["spB"]
    addA, addB = st["addA"], st["addB"]
    open_ps = {}

    for bi, (s, q0, nch) in enumerate(st["batches"]):
        gi = nch * P
        if pay_dram is not None:
            g = cx.gsp.tile([P, (GI_MAX // P) * D], BF, tag="gs", name="gs")
            nc.sync.dma_start(out=g[:, :nch * D], in_=pay_dram[:, q0 * D:(q0 + nch) * D])
        else:
            idx_t = cx.idxp.tile([128, GI_MAX // 16], I16, tag="gidx", name="gidx")
            nc.sync.dma_start(out=idx_t[:, :gi // 16],
                              in_=idx_dram[:, q0 * 8:(q0 + nch) * 8])
            g = cx.gp.tile([P, (GI_MAX // P) * D], F32, tag="gg", name="gg")
            # split into two sub-gathers on adjacent queues for deeper
            # SWDGE descriptor-generation overlap
            halves = [(0, nch)] if nch <= 1 else [(0, nch // 2), (nch // 2, nch)]
            for (h0, h1) in halves:
                nc.gpsimd.dma_gather(
                    out_ap=g[:, h0 * D:h1 * D].rearrange("p (c d) -> p c d", c=h1 - h0),
                    in_ap=src_tables[s],
                    idxs_ap=idx_t[:, h0 * 8:h1 * 8],
                    num_idxs=(h1 - h0) * P,
                    num_idxs_reg=(h1 - h0) * P,
                    elem_size=D,
                    single_packet=False,
                    queue_num=cx.qrr % 4,
                )
                cx.qrr += 1
        gv = cx.gvp.tile([P, (GI_MAX // P) * D], BF, tag="gv", name="gv")
        mul_eng = nc.vector
        mul_eng.tensor_mul(
            gv[:, :nch * D].rearrange("p (c d) -> p c d", c=nch),
            g[:, :nch * D].rearrange("p (c d) -> p c d", c=nch),
            val_sb[:, q0:q0 + nch].to_broadcast([P, nch, D]),
        )
        sel = {0: None, 1: None}
        for half, hmask, lr_sb in ((0, hasA, lrA_sb), (1, hasB, lrB_sb)):
            for (ra, rb) in _runs(hmask[q0:q0 + nch]):
                if sel[half] is None:
                    sel[half] = cx.selp.tile([P, (GI_MAX // P) * P], BF,
                                             tag=f"sel{half}", name=f"sel{half}")
                ln = rb - ra
                nc.vector.tensor_tensor(
                    out=sel[half][:, ra * P:rb * P].rearrange("p (c j) -> p c j", c=ln),
                    in0=cx.iota_bf[:].rearrange("p (o j) -> p o j", o=1).to_broadcast([P, ln, P]),
                    in1=lr_sb[:, q0 + ra:q0 + rb].to_broadcast([P, ln, P]),
                    op=ALU.is_equal)
        for k in range(nch):
            q = q0 + k
            w = int(ch_w[q])
            for half, hm, stX, spX, adX in ((0, hasA, stA, spA, addA),
                                            (1, hasB, stB, spB, addB)):
                if not hm[q]:
                    continue
                wf = 2 * w + half
                if stX[q]:
                    open_ps[wf] = cx.psp.tile(
                        [P, D], F32, space="PSUM",
                        tag=f"pseg{half}", name=f"pseg{half}", bufs=2)
                pt = open_ps[wf]
                nc.tensor.matmul(out=pt[:],
                                 lhsT=sel[half][:, k * P:(k + 1) * P],
                                 rhs=gv[:, k * D:(k + 1) * D],
                                 start=bool(stX[q]), stop=bool(spX[q]))
                if spX[q]:
                    t, sl = raw_put(wf)
                    if adX[q]:
                        nc.vector.tensor_add(t[:, sl], t[:, sl], pt[:])
                    else:
                        nc.scalar.activation(t[:, sl], pt[:], AF.Copy)
                    del open_ps[wf]
        if ag_after and bi in ag_after:
            for go in ag_after[bi]:
                go()
    assert not open_ps, f"{name}: unclosed psum groups {list(open_ps)}"


def emit_epilogue(cx, blocks, acc_sb, nprefix):
    """acc[:, w] += raw[:, w]/max(||raw_w||,1e-12) for windows 0..nprefix-1.
    blocks: list of (tile, nwin_in_tile)."""
    nc = cx.nc
    done = 0
    for (t, bw) in blocks:
        off = 0
        while off < bw and done < nprefix:
            ng = min(32, bw - off, nprefix - done)
            sl = slice(off * D, (off + ng) * D)
            sq = cx.ep.tile([P, 32 * D], F32, tag="ep_sq", name="ep_sq")
            nc.vector.tensor_mul(sq[:, :ng * D], t[:, sl], t[:, sl])
            ss = cx.ep.tile([P, 32], F32, tag="ep_ss", name="ep_ss")
            nc.vector.reduce_sum(ss[:, :ng], sq[:, :ng * D].rearrange("p (w d) -> p w d", w=ng),
                                 axis=mybir.AxisListType.X)
            snorm = cx.ep.tile([P, 32], F32, tag="ep_sn", name="ep_sn")
            nc.scalar.activation(snorm[:, :ng], ss[:, :ng], AF.Sqrt)
            nc.vector.tensor_scalar_max(snorm[:, :ng], snorm[:, :ng], 1e-12)
            rn = cx.ep.tile([P, 32], F32, tag="ep_rn", name="ep_rn")
            nc.vector.reciprocal(rn[:, :ng], snorm[:, :ng])
            contrib = cx.ep.tile([P, 32 * D], F32, tag="ep_ct", name="ep_ct")
            nc.vector.tensor_mul(
                contrib[:, :ng * D].rearrange("p (w d) -> p w d", w=ng),
                t[:, sl].rearrange("p (w d) -> p w d", w=ng),
                rn[:, :ng].to_broadcast([P, ng, D]),
            )
            nc.vector.tensor_add(acc_sb[:, done * D:(done + ng) * D],
                                 acc_sb[:, done * D:(done + ng) * D],
                                 contrib[:, :ng * D])
            done += ng
            off += ng


def indirect_gather_rows(cx, out_sb, table_ap, idx_sb, ncols):
    nc = cx.nc
    for k in range(ncols):
        nc.gpsimd.indirect_dma_start(
            out=out_sb[:, k * D:(k + 1) * D],
            out_offset=None,
            in_=table_ap,
            in_offset=bass.IndirectOffsetOnAxis(ap=idx_sb[:, k:k + 1], axis=0),
        )


def normalize_rows(cx, x_sb, ngroups, tag):
    nc = cx.nc
    sq = cx.lp.tile([P, ngroups * D], F32, tag=f"{tag}_sq")
    nc.vector.tensor_mul(sq[:], x_sb[:, :ngroups * D], x_sb[:, :ngroups * D])
    ss = cx.lp.tile([P, ngroups], F32, tag=f"{tag}_ss")
    nc.vector.reduce_sum(ss[:], sq[:].rearrange("p (w d) -> p w d", w=ngroups),
                         axis=mybir.AxisListType.X)
    sn = cx.lp.tile([P, ngroups], F32, tag=f"{tag}_sn")
    nc.scalar.activation(sn[:], ss[:], AF.Sqrt)
    nc.vector.tensor_scalar_max(sn[:], sn[:], 1e-12)
    rn = cx.lp.tile([P, ngroups], F32, tag=f"{tag}_rn")
    nc.vector.reciprocal(rn[:], sn[:])
    nc.vector.tensor_mul(
        x_sb[:, :ngroups * D].rearrange("p (w d) -> p w d", w=ngroups),
        x_sb[:, :ngroups * D].rearrange("p (w d) -> p w d", w=ngroups),
        rn[:].to_broadcast([P, ngroups, D]),
    )


def rowdot(cx, a_sb, b_sb, out_sb, ngroups, tag):
    nc = cx.nc
    t = cx.lp.tile([P, ngroups * D], F32, tag=f"{tag}_t")
    nc.vector.tensor_mul(t[:], a_sb[:, :ngroups * D], b_sb[:, :ngroups * D])
    nc.vector.reduce_sum(out_sb[:, :ngroups], t[:].rearrange("p (w d) -> p w d", w=ngroups),
                         axis=mybir.AxisListType.X)


def transpose_groups(cx, src_sb, ngroups, tag):
    nc = cx.nc
    out = cx.lp.tile([P, ngroups * P], F32, tag=f"{tag}_T")
    for g in range(ngroups):
        pt = cx.psp.tile([P, P], F32, space="PSUM", tag="tr_ps", bufs=1)
        nc.tensor.transpose(out=pt[:D, :P], in_=src_sb[:, g * D:(g + 1) * D],
                            identity=cx.ident[:])
        nc.vector.tensor_copy(out[:D, g * P:(g + 1) * P], pt[:D, :P])
    return out


def build(pp):
    dims = pp["dims"]
    R_acc1, R_f11 = dims["R_acc1"], dims["R_f11"]
    R_acc2, R_f12 = dims["R_acc2"], dims["R_f12"]
    RB = dims["RB"]
    blks_f11, blks_f12 = dims["blks_f11"], dims["blks_f12"]
    nsrc_ag = dims["nsrc_ag"]
    st = pp["streams"]

    nc = bacc.Bacc("TRN2", target_bir_lowering=False, debug=False,
                   num_devices=NCORES, num_swdge_queues=4)
    cx = Ctx()
    cx.nc = nc
    cx.qrr = 0
    cx.seli = 0

    # ---- dram inputs
    f0_acc_il_t = nc.dram_tensor("f0_acc_il", [R_acc1, D], F32, kind="ExternalInput")
    f0_acc_bl_t = nc.dram_tensor("f0_acc_bl", [R_acc2, D], F32, kind="ExternalInput")
    g_in = {}
    for nm in ("il1", "bl1", "il2", "bl2", "ag"):
        s = st[nm]
        tch = s["tch"]
        d = dict(
            lrA=nc.dram_tensor(f"{nm}_lrA", [128, tch], BF, kind="ExternalInput"),
            lrB=nc.dram_tensor(f"{nm}_lrB", [128, tch], BF, kind="ExternalInput"),
            val=nc.dram_tensor(f"{nm}_val", [128, tch],
                               BF if nm in ("il1", "bl1") else F32,
                               kind="ExternalInput"),
        )
        if nm in ("il1", "bl1"):
            d["pay"] = nc.dram_tensor(f"{nm}_pay", [128, tch * D], BF, kind="ExternalInput")
        else:
            d["idx"] = nc.dram_tensor(f"{nm}_idx", [128, tch * 8], I16, kind="ExternalInput")
        g_in[nm] = d
    debug = bool(int(os.environ.get("DSCBR_DEBUG", "0")))
    lidx = {k: nc.dram_tensor(f"loss_{k}", [128, v.shape[1]], I32, kind="ExternalInput")
            for k, v in pp["loss"][0].items()}
    aug_in = {k: nc.dram_tensor(k, [128, 16], I32, kind="ExternalInput")
              for k in ("aug_u_bl", "aug_b0_bl", "aug_b0_il")}
    out_t = nc.dram_tensor("out", [1, 2], F32, kind="ExternalOutput")

    with tile.TileContext(nc) as tc:
        cx.tc = tc
        es = []
        def pool(name, bufs, **kw):
            p = tc.tile_pool(name=name, bufs=bufs, **kw)
            es.append(p)
            return p.__enter__()
        cx.psp = pool("psum", 1, space="PSUM")
        cx.dramp = pool("dram", 1, space="DRAM")
        cx.cp = pool("const", 1)

        iota_i = cx.cp.tile([P, P], I32)
        nc.gpsimd.iota(iota_i[:], pattern=[[1, P]], base=0, channel_multiplier=0)
        cx.iota_bf = cx.cp.tile([P, P], BF)
        nc.vector.tensor_copy(cx.iota_bf[:], iota_i[:])
        cx.ident = cx.cp.tile([P, P], F32)
        make_identity(nc, cx.ident[:])
        ones_col = cx.cp.tile([P, 1], F32)
        nc.vector.memset(ones_col[:], 1.0)

        # ---------- scoped pools for the SpMM phases ----------
        es2 = []
        def pool2(name, bufs, **kw):
            p = tc.tile_pool(name=name, bufs=bufs, **kw)
            es2.append(p)
            return p.__enter__()
        cx.gsp = pool2("gstream", 6)
        cx.gp = pool2("gather", 8)
        cx.gvp = pool2("gval", 3)
        cx.idxp = pool2("gidx", 8)
        cx.selp = pool2("sel", 2)
        cx.ep = pool2("epil", 1)
        cx.mp = pool2("meta", 1)
        cx.accp = pool2("accs", 1)

        cx.meta = {}
        def load_meta(nm):
            s = st[nm]
            tch = s["tch"]
            vt = BF if nm in ("il1", "bl1") else F32
            lrA = cx.mp.tile([128, tch], BF, tag=f"{nm}_lrA", name=f"{nm}_lrA")
            lrB = cx.mp.tile([128, tch], BF, tag=f"{nm}_lrB", name=f"{nm}_lrB")
            vv = cx.mp.tile([128, tch], vt, tag=f"{nm}_vv", name=f"{nm}_vv")
            nc.sync.dma_start(out=lrA[:], in_=g_in[nm]["lrA"][:])
            nc.sync.dma_start(out=lrB[:], in_=g_in[nm]["lrB"][:])
            nc.sync.dma_start(out=vv[:], in_=g_in[nm]["val"][:])
            cx.meta[nm] = (lrA, lrB, vv)

        # acc buffers
        nacc1, nacc2 = R_acc1 // P, R_acc2 // P
        acc_il = cx.accp.tile([P, nacc1 * D], F32, tag="acc_il", name="acc_il")
        nc.sync.dma_start(out=acc_il[:].rearrange("p (w d) -> p w d", w=nacc1),
                          in_=f0_acc_il_t[:].rearrange("(w p) d -> p w d", p=P))
        acc_bl = cx.accp.tile([P, nacc2 * D], F32, tag="acc_bl", name="acc_bl")
        nc.sync.dma_start(out=acc_bl[:].rearrange("p (w d) -> p w d", w=nacc2),
                          in_=f0_acc_bl_t[:].rearrange("(w p) d -> p w d", p=P))

        # raw block tiles (32 windows each), shared by il1/bl1
        nblk = max(len(blks_f11), len(blks_f12))
        def raw_blocks(blks):
            tiles = []
            for i, bs in enumerate(blks):
                t = cx.accp.tile([P, 32 * D], F32, tag=f"rawblk{i}", name=f"rawblk{i}")
                tiles.append((t, bs // P))
            return tiles
        def raw_put_blocks(tiles):
            def put(wf):
                return tiles[wf // 32][0], slice((wf % 32) * D, (wf % 32 + 1) * D)
            return put

        # collective helper
        def ag_pair(nm, rows_in, rows_out):
            ain = cx.dramp.tile([rows_in, D], F32, tag=f"{nm}_agin", name=f"{nm}_agin")
            aout = cx.dramp.tile([rows_out, D], F32, addr_space="Shared",
                                 tag=f"{nm}_agout", name=f"{nm}_agout")
            return ain, aout

        def emit_l1(nm, blks, pay_t, last_batch_of_blk):
            tiles = raw_blocks(blks)
            outs = []
            ag_after = {}
            for i, bs in enumerate(blks):
                ain, aout = ag_pair(f"{nm}b{i}", bs, bs * NCORES)
                outs.append(aout)
                def mk(i=i, bs=bs, ain=ain, aout=aout):
                    def go():
                        t, nw = tiles[i]
                        nc.sync.dma_start(
                            out=ain[:].rearrange("(w p) d -> p w d", p=P),
                            in_=t[:, :nw * D].rearrange("p (w d) -> p w d", w=nw))
                        nc.gpsimd.collective_compute(
                            "AllGather", ALU.bypass,
                            replica_groups=[list(range(NCORES))],
                            ins=[ain[:].opt()], outs=[aout[:].opt()])
                    return go
                ag_after.setdefault(last_batch_of_blk[i], []).append(mk())
            emit_graph(cx, nm, st[nm], None, pay_t, None,
                       raw_put_blocks(tiles), ag_after=ag_after)
            return tiles, outs

        def last_batches(s, blks):
            """batch index after which each block's drains are complete."""
            nbat = len(s["batches"])
            out = []
            for i in range(len(blks)):
                wlo, whi = (sum(b // P for b in blks[:i]),
                            sum(b // P for b in blks[:i + 1]))
                lb = 0
                for bi, (sg, q0, nch) in enumerate(s["batches"]):
                    for q in range(q0, q0 + nch):
                        w = int(s["ch_w"][q])
                        if wlo <= 2 * w < whi or wlo <= 2 * w + 1 < whi:
                            if s["wp_last_q"][w] == q:
                                lb = bi
                out.append(lb)
            return out

        # wp_last_q helper array on streams
        for nm in ("il1", "bl1", "il2", "bl2", "ag"):
            s = st[nm]
            wpl = np.full(s["nwp"], -1, np.int64)
            for w in range(s["nwp"]):
                wpl[w] = max(s["lastq"][2 * w], s["lastq"][2 * w + 1])
            s["wp_last_q"] = wpl

        # ---------------- il1 ----------------
        for nm in ("il1", "bl1", "il2", "bl2", "ag"):
            load_meta(nm)
        il1_tiles, f1_il_blks = emit_l1("il1", blks_f11, g_in["il1"]["pay"],
                                        last_batches(st["il1"], blks_f11))
        if debug:
            raw_dump = nc.dram_tensor("dbg_raw_il1b0", [blks_f11[0], D], F32,
                                      kind="ExternalOutput")
            t0, nw0 = il1_tiles[0]
            nc.sync.dma_start(out=raw_dump[:].rearrange("(w p) d -> p w d", p=P),
                              in_=t0[:, :nw0 * D].rearrange("p (w d) -> p w d", w=nw0))
        emit_epilogue(cx, il1_tiles, acc_il, nacc1)

        # ---------------- bl1 ----------------
        bl1_tiles, f1_bl_blks = emit_l1("bl1", blks_f12, g_in["bl1"]["pay"],
                                        last_batches(st["bl1"], blks_f12))
        emit_epilogue(cx, bl1_tiles, acc_bl, nacc2)

        # ---------------- il2 ----------------
        raw2 = cx.accp.tile([P, nacc1 * D], F32, tag="raw2", name="raw2")
        def raw2_put(wf):
            return raw2, slice(wf * D, (wf + 1) * D)
        emit_graph(cx, "il2", st["il2"], [t[:] for t in f1_il_blks], None,
                   g_in["il2"]["idx"], raw2_put)
        emit_epilogue(cx, [(raw2, nacc1)], acc_il, nacc1)
        acc_il_in, acc_il_full = ag_pair("accil", R_acc1, R_acc1 * NCORES)
        nc.sync.dma_start(out=acc_il_in[:].rearrange("(w p) d -> p w d", p=P),
                          in_=acc_il[:].rearrange("p (w d) -> p w d", w=nacc1))
        nc.gpsimd.collective_compute(
            "AllGather", ALU.bypass, replica_groups=[list(range(NCORES))],
            ins=[acc_il_in[:].opt()], outs=[acc_il_full[:].opt()])

        # ---------------- bl2 ----------------
        raw2b = cx.accp.tile([P, nacc2 * D], F32, tag="raw2b", name="raw2b")
        def raw2b_put(wf):
            return raw2b, slice(wf * D, (wf + 1) * D)
        emit_graph(cx, "bl2", st["bl2"], [t[:] for t in f1_bl_blks], None,
                   g_in["bl2"]["idx"], raw2b_put)
        emit_epilogue(cx, [(raw2b, nacc2)], acc_bl, nacc2)
        acc_bl_in, acc_bl_full = ag_pair("accbl", R_acc2, R_acc2 * NCORES)
        nc.sync.dma_start(out=acc_bl_in[:].rearrange("(w p) d -> p w d", p=P),
                          in_=acc_bl[:].rearrange("p (w d) -> p w d", w=nacc2))
        nc.gpsimd.collective_compute(
            "AllGather", ALU.bypass, replica_groups=[list(range(NCORES))],
            ins=[acc_bl_in[:].opt()], outs=[acc_bl_full[:].opt()])

        # ---------------- agg ----------------
        nwB = RB // P
        rawag = cx.accp.tile([P, nwB * D], F32, tag="rawag", name="rawag")
        def rawag_put(wf):
            return rawag, slice(wf * D, (wf + 1) * D)
        acc_il_ap = acc_il_full[:]
        ag_tables = [acc_il_ap[s * SRC_WIN: min((s + 1) * SRC_WIN, R_acc1 * NCORES), :]
                     for s in range(nsrc_ag)]
        emit_graph(cx, "ag", st["ag"], ag_tables, None, g_in["ag"]["idx"], rawag_put)
        ilb_in, ilb_full = ag_pair("ilb", RB, RB * NCORES)
        nc.sync.dma_start(out=ilb_in[:].rearrange("(w p) d -> p w d", p=P),
                          in_=rawag[:, :nwB * D].rearrange("p (w d) -> p w d", w=nwB))
        nc.gpsimd.collective_compute(
            "AllGather", ALU.bypass, replica_groups=[list(range(NCORES))],
            ins=[ilb_in[:].opt()], outs=[ilb_full[:].opt()])

        if debug:
            for nm, t, rows in (("dbg_acc_il", acc_il_full, R_acc1 * NCORES),
                                ("dbg_acc_bl", acc_bl_full, R_acc2 * NCORES),
                                ("dbg_ilb", ilb_full, RB * NCORES),
                                ("dbg_f1il0", f1_il_blks[0], blks_f11[0] * NCORES)):
                o = nc.dram_tensor(nm, [rows, D], F32, kind="ExternalOutput")
                nc.sync.dma_start(out=o[:], in_=t[:])

        for p in reversed(es2):
            p.__exit__(None, None, None)
        cx.lp = pool("loss", 1)

        # ---------------- loss ----------------
        bsh = BATCH // NCORES          # 256
        ng = bsh // P                  # 2
        lidx_sb = {}
        for k, t in lidx.items():
            s = cx.lp.tile([128, t.shape[1]], I32, tag=f"li_{k}")
            nc.sync.dma_start(out=s[:], in_=t[:])
            lidx_sb[k] = s
        for k, t in aug_in.items():
            s = cx.lp.tile([128, 16], I32, tag=f"li_{k}")
            nc.sync.dma_start(out=s[:], in_=t[:])
            lidx_sb[k] = s

        def gather(tag, table, idxk, ncols):
            sb = cx.lp.tile([P, ncols * D], F32, tag=tag)
            indirect_gather_rows(cx, sb, table, lidx_sb[idxk], ncols)
            return sb
        pos_u_il = gather("pos_u_il", acc_il_full[:], "u_il", ng)
        pos_u_bl = gather("pos_u_bl", acc_bl_full[:], "u_bl", ng)
        b_il0 = gather("b_il0", ilb_full[:], "b_il0", ng)
        b_il1 = gather("b_il1", ilb_full[:], "b_il1", ng)
        b_bl0 = gather("b_bl0", acc_bl_full[:], "b_bl0", ng)
        b_bl1 = gather("b_bl1", acc_bl_full[:], "b_bl1", ng)
        aug_u = gather("aug_u", acc_bl_full[:], "aug_u_bl", 16)
        aug_b = gather("aug_b", acc_bl_full[:], "aug_b0_bl", 16)
        # -- bpr
        pr0 = cx.lp.tile([P, ng], F32, tag="pr0")
        pr1 = cx.lp.tile([P, ng], F32, tag="pr1")
        tmp = cx.lp.tile([P, ng], F32, tag="prt")
        rowdot(cx, pos_u_il, b_il0, pr0, ng, "d0")
        rowdot(cx, pos_u_bl, b_bl0, tmp, ng, "d1")
        nc.vector.tensor_add(pr0[:], pr0[:], tmp[:])
        rowdot(cx, pos_u_il, b_il1, pr1, ng, "d2")
        rowdot(cx, pos_u_bl, b_bl1, tmp, ng, "d3")
        nc.vector.tensor_add(pr1[:], pr1[:], tmp[:])
        x = cx.lp.tile([P, ng], F32, tag="bprx")
        nc.vector.tensor_tensor(out=x[:], in0=pr1[:], in1=pr0[:], op=ALU.subtract)
        negx = cx.lp.tile([P, ng], F32, tag="bprnx")
        nc.vector.tensor_scalar_mul(negx[:], x[:], -1.0)
        nax = cx.lp.tile([P, ng], F32, tag="bprax")
        nc.vector.tensor_tensor(out=nax[:], in0=x[:], in1=negx[:], op=ALU.min)
        e = cx.lp.tile([P, ng], F32, tag="bpre")
        nc.scalar.activation(e[:], nax[:], AF.Exp)
        nc.vector.tensor_scalar_add(e[:], e[:], 1.0)
        l1p = cx.lp.tile([P, ng], F32, tag="bprl")
        nc.scalar.activation(l1p[:], e[:], AF.Ln)
        sp = cx.lp.tile([P, ng], F32, tag="bprsp")
        nc.vector.tensor_scalar_max(sp[:], x[:], 0.0)
        nc.vector.tensor_add(sp[:], sp[:], l1p[:])

        normalize_rows(cx, aug_u, 16, "nau")
        normalize_rows(cx, aug_b, 16, "nab")
        normalize_rows(cx, pos_u_il, ng, "npu")
        my_pos_b = cx.lp.tile([P, ng * D], F32, tag="my_pb")
        nc.vector.tensor_copy(my_pos_b[:], b_il0[:, :ng * D])
        normalize_rows(cx, my_pos_b, ng, "npb")

        part = cx.lp.tile([P, 4], F32, tag="parts")
        nc.vector.memset(part[:], 0.0)
        nc.vector.reduce_sum(part[:, 0:1], sp[:].rearrange("p (w d) -> p w d", w=1),
                             axis=mybir.AxisListType.X)

        def closs_partial(pos_my, aug_full, aug_my_cols, out_col):
            posT = transpose_groups(cx, pos_my, ng, "pT")
            augT = transpose_groups(cx, aug_full, 16, "aT")
            ps = cx.lp.tile([P, ng], F32, tag="ps")
            rowdot(cx, pos_my, aug_my_cols, ps, ng, f"psd{out_col}")
            lse = cx.lp.tile([P, ng], F32, tag="lse")
            for g in range(ng):
                ttl_ps = cx.psp.tile([P, 512], F32, space="PSUM", tag="ttl", bufs=1)
                ttl = cx.lp.tile([P, BATCH], F32, tag="ttl")
                for nb_ in range(BATCH // 512):
                    nc.tensor.matmul(
                        out=ttl_ps[:, :512],
                        lhsT=posT[:D, g * P:(g + 1) * P],
                        rhs=augT[:D, nb_ * 512:(nb_ + 1) * 512],
                        start=True, stop=True)
                    nc.vector.tensor_copy(ttl[:, nb_ * 512:(nb_ + 1) * 512], ttl_ps[:, :512])
                mx = cx.lp.tile([P, 1], F32, tag="mx")
                nc.vector.reduce_max(mx[:], ttl[:].rearrange("p (w d) -> p w d", w=1),
                                     axis=mybir.AxisListType.X)
                nmx = cx.lp.tile([P, 1], F32, tag="nmx")
                nc.vector.tensor_scalar_mul(nmx[:], mx[:], -4.0)
                ex = cx.lp.tile([P, BATCH], F32, tag="ex")
                se = cx.lp.tile([P, 1], F32, tag="se")
                nc.scalar.activation(ex[:], ttl[:], AF.Exp, bias=nmx[:, :1], scale=4.0,
                                     accum_out=se[:, :1])
                ln = cx.lp.tile([P, 1], F32, tag="ln")
                nc.scalar.activation(ln[:], se[:], AF.Ln)
                m4 = cx.lp.tile([P, 1], F32, tag="m4")
                nc.vector.tensor_scalar_mul(m4[:], mx[:], 4.0)
                nc.vector.tensor_add(lse[:, g:g + 1], ln[:], m4[:])
            t4 = cx.lp.tile([P, ng], F32, tag="t4")
            nc.vector.tensor_scalar_mul(t4[:], ps[:], 4.0)
            nc.vector.tensor_tensor(out=t4[:], in0=t4[:], in1=lse[:], op=ALU.subtract)
            nc.vector.reduce_sum(part[:, out_col:out_col + 1],
                                 t4[:].rearrange("p (w d) -> p w d", w=1),
                                 axis=mybir.AxisListType.X)

        aug_u_my = gather("aug_u_my", acc_bl_full[:], "u_bl", ng)
        normalize_rows(cx, aug_u_my, ng, "naum")
        aug_b_my = gather("aug_b_my", acc_bl_full[:], "b_bl0", ng)
        normalize_rows(cx, aug_b_my, ng, "nabm")
        closs_partial(pos_u_il, aug_u, aug_u_my, 1)
        closs_partial(my_pos_b, aug_b, aug_b_my, 2)

        pp_ps = cx.psp.tile([P, 4], F32, space="PSUM", tag="ppps", bufs=1)
        nc.tensor.matmul(out=pp_ps[:1, :4], lhsT=ones_col[:], rhs=part[:],
                         start=True, stop=True)
        psum_sb = cx.lp.tile([1, 4], F32, tag="psums")
        nc.vector.tensor_copy(psum_sb[:], pp_ps[:1, :4])
        ar_in = cx.dramp.tile([1, 4], F32, tag="ar_in")
        ar_out = cx.dramp.tile([1, 4], F32, addr_space="Shared", tag="ar_out")
        nc.sync.dma_start(out=ar_in[:], in_=psum_sb[:])
        nc.gpsimd.collective_compute(
            "AllReduce", ALU.add, replica_groups=[list(range(NCORES))],
            ins=[ar_in[:].opt()], outs=[ar_out[:].opt()])
        fin = cx.lp.tile([1, 4], F32, tag="fin")
        nc.sync.dma_start(out=fin[:], in_=ar_out[:])
        res = cx.lp.tile([1, 2], F32, tag="res")
        nc.vector.tensor_scalar_mul(res[:, 0:1], fin[:, 0:1], 1.0 / BATCH)
        t = cx.lp.tile([1, 1], F32, tag="rt")
        nc.vector.tensor_add(t[:], fin[:, 1:2], fin[:, 2:3])
        nc.vector.tensor_scalar_mul(res[:, 1:2], t[:], -0.5 / BATCH)
        nc.sync.dma_start(out=out_t[:], in_=res[:])

        for p in reversed(es):
            p.__exit__(None, None, None)
    nc.compile()
    return nc


# ---------------------------------------------------------------- entry point

def _install_ntff_hook():
    if "antenv.axon_hooks" in sys.modules:
        return
    try:
        mod = types.ModuleType("antenv.axon_hooks")
        _hook = [None]
        mod.set_axon_ntff_profile_hook = lambda h: _hook.__setitem__(0, h)
        mod.get_axon_ntff_profile_hook = lambda: _hook[0]
        sys.modules["antenv.axon_hooks"] = mod
        import antenv
        antenv.axon_hooks = mod
        from trn_agent_boot.trn_boot import _ntff_profile_via_ctypes
        hook = _ntff_profile_via_ctypes("/opt/axon/libaxon_pjrt.so")
        if hook is not None:
            mod.set_axon_ntff_profile_hook(hook)
    except Exception:
        pass


def make_in_maps(pp):
    import ml_dtypes
    maps = []
    st = pp["streams"]
    for c in range(NCORES):
        m = {
            "f0_acc_il": pp["f0_acc"]["il"][c],
            "f0_acc_bl": pp["f0_acc"]["bl"][c],
            "aug_u_bl": pp["aug_u_bl"], "aug_b0_bl": pp["aug_b0_bl"],
            "aug_b0_il": pp["aug_b0_il"],
        }
        for nm in ("il1", "bl1", "il2", "bl2", "ag"):
            s = st[nm]
            m[f"{nm}_lrA"] = np.ascontiguousarray(
                s["lrA"][c].reshape(-1, P).T).astype(ml_dtypes.bfloat16)
            m[f"{nm}_lrB"] = np.ascontiguousarray(
                s["lrB"][c].reshape(-1, P).T).astype(ml_dtypes.bfloat16)
            vals = np.ascontiguousarray(s["val"][c].reshape(-1, P).T)
            if nm in ("il1", "bl1"):
                m[f"{nm}_val"] = vals.astype(ml_dtypes.bfloat16)
                m[f"{nm}_pay"] = pp["pay"][nm][c]
            else:
                m[f"{nm}_val"] = vals
                m[f"{nm}_idx"] = wrap_idx16(s["srcv"][c].astype(np.int16))
        for k, v in pp["loss"][c].items():
            m[f"loss_{k}"] = v
        maps.append(m)
    return maps


_CACHE = {}


def kernel(**inputs) -> np.ndarray:
    _install_ntff_hook()
    pp = preprocess(inputs)
    key = tuple(sorted((k, v) for k, v in pp["dims"].items()
                       if isinstance(v, int)))
    if key not in _CACHE:
        _CACHE[key] = build(pp)
    nc = _CACHE[key]
    in_maps = make_in_maps(pp)
    trace = bool(int(os.environ.get("DSCBR_TRACE", "0")))
    res = run_bass_kernel_spmd(nc, in_maps, core_ids=list(range(NCORES)), trace=trace)
    if trace and res.exec_time_ns:
        print(f"HW exec time: {res.exec_time_ns} ns")
    kernel._last_results = res.results
    out = res.results[0]["out"].reshape(2).astype(np.float32)
    return out
